# revision 56
# baseline (speedup 1.0000x reference)
"""Trainium2 Bass kernel for multi-head causal attention.

Problem: B=2, S=2048, D=1024, H=16 heads (head_dim=64), fp32.
  q,k,v = x@Wq, x@Wk, x@Wv  (per-head split)
  scores = q@k^T, causal mask, softmax(scores/sqrt(64))
  out = (attn@v concat) @ Wo + bo

Sharding (8 cores): core c -> batch b=c//4, head group g=c%4 (4 heads).
Each core computes its 4 heads' attention plus the partial output
projection (row-parallel Wo); host sums 4 partials per batch and adds bo.

Layout strategy (zero on-device transposes):
 - x^T passed host-transposed (feature-major).
 - Q^T,K^T produced feature-major: (head_dim x tokens), two heads stacked
   per 128-partition tile; scores^T computed per 64-partition row group.
 - Both heads' scores^T tiles (k x q) land in one 2-bank PSUM tile so the
   causal mask add + exp run as single wide instructions. The exp'd
   bf16 tile is directly the PV stationary operand. V is token-major with
   an appended ones-column so the PV matmul also emits the softmax
   denominators.

Scheduling: the attention inner loop is ACT-bound - each k-chunk's exp
(~1.1us) exceeds the PE work for that chunk (~0.65us scores+PV). All
projection/output-projection matmuls are therefore decomposed into
single-matmul work items and PUMPED into the per-chunk PE idle windows
with an ns-budget pacer, so the tensor engine streams continuously while
the scalar engine streams exps. Emission-order rules learned from traces:
 - scores(j) is emitted BEFORE pv(j-2) every iteration: the exp-stream
   critical chain is exp(j-2) ->[sc slot]-> scores(j) -> exp(j); the PV
   pair's consumer is many chunks away.
 - WO chains carry an inflated pump cost so they spread out ~1 chain per
   1.5 js: a dense WO burst makes chain n's psum-slot wait (freed by
   chain n-2's staging copy) back up into the in-order PE queue ahead of
   the attention stream.
 - Cross-pair score prefill between the two drain PVs keeps ACT fed
   across pair boundaries; range-0 pairs emit all scores before any PV
   because the PVs wait on the late wv DMA.
 - gpsimd runs ONLY partition_broadcast: mixing in other ops thrashes
   LOAD_LIB/UNLOAD_LIB microcode swaps with multi-us stalls; gpsimd also
   cannot touch PSUM, so all PSUM->SBUF staging is DVE (+ scalar for the
   o=0 WO half).
Softmax normalization (reciprocal of the ones-column denominators +
gpsimd partition broadcast + DVE multiply) runs off the critical path,
with a PE-row-broadcast fast path for the tail-critical final pair; the
final-range WO chains alternate into the idle scores psum tag for a
4-slot tail pipeline.

PSUM budget (8 banks): scores 2 tiles x 2 banks, PV accumulators 2 x 1,
projection/WO chains 2 x 1.
Matmul operands are bf16 (1 cycle/row PE rate); every accumulation and
the softmax normalization stay fp32 in PSUM.
"""

import sys

sys.path.insert(0, "/opt/trn_rl_repo")

from collections import deque

import ml_dtypes
import numpy as np

import concourse.bass as bass  # noqa: F401
import concourse.tile as tile
from concourse import bacc, bass_utils, mybir

F32 = mybir.dt.float32
MMDT = mybir.dt.bfloat16
NPDT = ml_dtypes.bfloat16
EXPF = mybir.ActivationFunctionType.Exp

B, S, D, H, HD = 2, 2048, 1024, 16, 64
N_CORES = 8
HPC = 4            # heads per core
GW = HPC * HD      # head-group width per core = 256
SCALE = 1.0 / np.sqrt(HD)
NEG = -1.0e30

NT = S // 512      # 4 q/t ranges of 512
NC = D // 128      # 8 contraction chunks for projections
NJ = S // 128      # 16 k-chunks

_CACHE = {}
LAST_RESULTS = None


def _maybe_install_trace_hook():
    """If BASS_TRACE is set, bass_utils needs antenv.axon_hooks (absent in
    this image). Install it from trn_boot when possible; otherwise disable
    tracing so the run still works."""
    import os

    if not os.environ.get("BASS_TRACE"):
        return
    try:
        import antenv.axon_hooks  # noqa: F401
        return
    except ImportError:
        pass
    try:
        import types

        from trn_agent_boot.trn_boot import _ntff_profile_via_ctypes

        hook = _ntff_profile_via_ctypes("/opt/axon/libaxon_pjrt.so")
        mod = types.ModuleType("antenv.axon_hooks")
        mod.get_axon_ntff_profile_hook = lambda: hook
        mod.set_axon_ntff_profile_hook = lambda h: None
        import antenv

        sys.modules["antenv.axon_hooks"] = mod
        antenv.axon_hooks = mod
    except Exception:
        os.environ["BASS_NEVER_TRACE"] = "1"


def _build():
    nc = bacc.Bacc("TRN2", target_bir_lowering=False, debug=False)

    # x is passed host-rearranged range-major: row 128*r+p holds the NC=8
    # contraction chunks for q-range r, each 512 tokens contiguous, so one
    # per-range DMA is 128 descriptors x 8KB.
    xT = nc.dram_tensor("xT", [(S // 512) * 128, (D // 128) * 512], MMDT,
                        kind="ExternalInput").ap()
    wq = nc.dram_tensor("wq", [128, D // 128 * GW], MMDT, kind="ExternalInput").ap()
    wk = nc.dram_tensor("wk", [128, D // 128 * GW], MMDT, kind="ExternalInput").ap()
    wv = nc.dram_tensor("wv", [128, D // 128 * GW], MMDT, kind="ExternalInput").ap()
    wo = nc.dram_tensor("wo", [128, GW // 128 * D], MMDT, kind="ExternalInput").ap()
    trid = nc.dram_tensor("tri", [128, 128], F32, kind="ExternalInput").ap()
    out = nc.dram_tensor("out", [S, D], MMDT, kind="ExternalOutput").ap()

    with tile.TileContext(nc) as tc, nc.allow_low_precision(reason="bf16 matmuls"):
        with (
            tc.tile_pool(name="const", bufs=1) as cpool,
            tc.tile_pool(name="xin", bufs=3) as xpool,
            tc.tile_pool(name="pt", bufs=32) as ppool,
            tc.tile_pool(name="small", bufs=4) as spool,
            tc.tile_pool(name="psum", bufs=1, space="PSUM") as psum,
        ):
            # ---- persistent tiles ----
            wq_sb = cpool.tile([128, NC, GW], MMDT)
            wk_sb = cpool.tile([128, NC, GW], MMDT)
            wv_sb = cpool.tile([128, NC, GW], MMDT)
            wo_sb = cpool.tile([128, 2, D], MMDT)

            QT = cpool.tile([128, 2, S], MMDT)   # [:, pair, t] feature-major
            KT = cpool.tile([128, 2, S], MMDT)
            Vt = cpool.tile([128, NJ, HPC * 65], MMDT)  # token-major + ones col
            ctxT = cpool.tile([128, 2, S], MMDT)

            # PE warmup: stream constant data through the tensor engine while
            # the first DMAs land, so the DVFS activity monitor ramps the PE
            # clock to max before real matmuls begin. One accumulation group
            # so no inter-instruction semaphores serialize it.
            warm_sb = cpool.tile([128, 512], MMDT, name="warm")
            nc.vector.memset(warm_sb[:], 0.125)
            warm_ps = psum.tile([128, 1024], F32, tag="sc", bufs=2)
            NWARM = 14
            for i in range(NWARM):
                nc.tensor.matmul(
                    warm_ps[:, 0:512], warm_sb[:, 0:128], warm_sb[:],
                    start=(i == 0), stop=(i == NWARM - 1),
                )
            # preload the Exp activation table during the DMA preamble so the
            # first real exp doesn't eat the lazy ACT_TABLE_LOAD.
            tbl = cpool.tile([1, 8], F32, name="tbl")
            nc.scalar.activation(tbl[:], warm_sb[0:1, 0:8], EXPF, scale=SCALE)

            # all-ones tile; row 64 is the stationary for the PE row-broadcast
            # of the softmax denominators (partition 64 -> partitions 0..63).
            ones128 = cpool.tile([128, 64], MMDT, name="ones128")
            nc.vector.memset(ones128[:], 1.0)

            # ones columns of V (col 64 of each 65-wide head slot)
            vt_ones = Vt[:, :, :].rearrange("p j (h u) -> p (j h) u", u=65)[:, :, 64:65]
            nc.vector.memset(vt_ones, 1.0)

            # triangular causal mask for the diagonal 128-block of scores^T:
            # keep (q - k >= 0) else -1e30   [partition = k, free = q]
            # (DMA emitted later, after the startup-critical wq/x transfers)
            tri = cpool.tile([128, 128], F32, name="tri")

            # broadcast view of tri over the two stacked heads (0-stride dim)
            tri_ap = tri[:]
            tri2 = bass.AP(
                tensor=tri_ap.tensor,
                offset=tri_ap.offset,
                ap=[list(tri_ap.ap[0]), [0, 2], list(tri_ap.ap[1])],
            )

            xts = {}

            def load_xt(r, split=False):
                xt = xpool.tile([128, NC, 512], MMDT, tag="xt")
                xv = xT[128 * r : 128 * (r + 1), :].rearrange(
                    "p (c t) -> p c t", t=512
                )
                if split:
                    # startup: land the first chunks earlier so the first
                    # projection matmuls can begin sooner.
                    nc.sync.dma_start(xt[:, 0:4, :], xv[:, 0:4, :])
                    nc.sync.dma_start(xt[:, 4:8, :], xv[:, 4:8, :])
                else:
                    nc.sync.dma_start(xt[:], xv)
                xts[r] = xt

            # ---------- filler work-item machinery ----------
            # Each item: (pe_cost_ns, fn). Markers gate force-drains so a
            # consumer can guarantee a producer chain has been emitted.
            fillers = deque()
            markers_done = set()

            def pump(budget):
                while fillers:
                    head = fillers[0]
                    if head[0] is None:
                        fillers.popleft()
                        markers_done.add(head[1])
                        continue
                    if head[0] > budget:
                        break
                    fillers.popleft()
                    head[1]()
                    budget -= head[0]
                return budget

            def drain_until(name):
                if name in markers_done:
                    return
                while fillers:
                    head = fillers.popleft()
                    if head[0] is None:
                        markers_done.add(head[1])
                        if head[1] == name:
                            return
                        continue
                    head[1]()

            def qk_items(r, w_sb, dst, o):
                st = {}
                def item(c):
                    def go():
                        if c == 0:
                            st["pm"] = psum.tile([128, 512], F32, tag="pj", bufs=2, name=f"pm{r}_{o}_{id(st)%97}")
                        nc.tensor.matmul(
                            st["pm"][:, :],
                            w_sb[:, c, 128 * o : 128 * (o + 1)],
                            xts[r][:, c, :],
                            start=(c == 0),
                            stop=(c == NC - 1),
                        )
                        if c == NC - 1:
                            # two half-copies: finer DVE granularity keeps
                            # the diagonal tri-adds from queuing behind a
                            # full-width copy.
                            nc.vector.tensor_copy(
                                dst[:, o, 512 * r : 512 * r + 256],
                                st["pm"][:, 0:256],
                            )
                            nc.vector.tensor_copy(
                                dst[:, o, 512 * r + 256 : 512 * (r + 1)],
                                st["pm"][:, 256:512],
                            )
                    return (216, go)
                return [item(c) for c in range(NC)]

            def v_items(r, tt):
                st = {}
                def item(c):
                    def go():
                        if c == 0:
                            st["pv"] = psum.tile([128, 512], F32, tag="pj", bufs=2, name=f"pvp{r}_{tt}")
                        nc.tensor.matmul(
                            st["pv"][:, 0:GW],
                            xts[r][:, c, 128 * tt : 128 * (tt + 1)],
                            wv_sb[:, c, :],
                            start=(c == 0),
                            stop=(c == NC - 1),
                        )
                        if c == NC - 1:
                            j = 4 * r + tt
                            vtv = Vt[:, j, :].rearrange(
                                "p (h u) -> p h u", u=65)
                            pvv = st["pv"][:, 0:GW].rearrange(
                                "p (h d) -> p h d", d=HD)
                            nc.vector.tensor_copy(
                                vtv[:, 0:2, 0:64], pvv[:, 0:2, :])
                            nc.vector.tensor_copy(
                                vtv[:, 2:4, 0:64], pvv[:, 2:4, :])
                    return (112, go)
                return [item(c) for c in range(NC)]

            def wo_items(r, qq, o, ptag="pj"):
                st = {}
                qt = 4 * r + qq
                def item(d):
                    def go():
                        if d == 0:
                            if ptag == "sc":
                                # tail chains borrow the (idle) scores psum
                                # tag for a deeper WO pipeline.
                                t = psum.tile([128, 1024], F32, tag="sc",
                                              bufs=2, name=f"posc{qt}_{o}")
                                st["po"] = t[:, 0:512]
                            else:
                                st["po"] = psum.tile(
                                    [128, 512], F32, tag="pj", bufs=2,
                                    name=f"po{qt}_{o}")[:, :]
                        nc.tensor.matmul(
                            st["po"],
                            ctxT[:, d, 128 * qt : 128 * (qt + 1)],
                            wo_sb[:, d, 512 * o : 512 * (o + 1)],
                            start=(d == 0), stop=(d == 1),
                        )
                        if d == 1:
                            # stage to SBUF (DMA cannot read PSUM), then DMA
                            # the bf16 partial out; host sums the partials.
                            # Copies alternate scalar/DVE to spread load.
                            ot = spool.tile(
                                [128, 512], MMDT, tag="ot", name=f"ot{qt}_{o}"
                            )
                            if o == 0:
                                nc.scalar.copy(ot[:], st["po"])
                            else:
                                nc.vector.tensor_copy(ot[:, 0:256], st["po"][:, 0:256])
                                nc.vector.tensor_copy(ot[:, 256:512], st["po"][:, 256:512])
                            nc.sync.dma_start(
                                out[128 * qt : 128 * (qt + 1),
                                    512 * o : 512 * (o + 1)],
                                ot[:],
                            )
                    # inflated pump cost: paces WO chains to ~one per 3-4
                    # j-iterations so a chain's psum-slot wait (freed by the
                    # previous chain's copy) never backs up into the in-order
                    # PE queue ahead of the attention stream.
                    return (450, go)
                return [item(d) for d in range(2)]

            def add_a_fillers(r):
                fillers.extend(qk_items(r, wq_sb, QT, 0))
                fillers.append((None, f"Q{r}o0"))
                fillers.extend(qk_items(r, wk_sb, KT, 0))
                fillers.append((None, f"K{r}o0"))
                for tt in range(4):
                    fillers.extend(v_items(r, tt))
                    fillers.append((None, f"V{r}t{tt}"))
                fillers.extend(qk_items(r, wq_sb, QT, 1))
                fillers.extend(qk_items(r, wk_sb, KT, 1))
                fillers.append((None, f"a{r}"))

            def add_c_fillers(r):
                for qq in range(4):
                    for o in range(2):
                        fillers.extend(wo_items(r, qq, o))

            # ---------- attention ----------
            pairs = [(r, p) for r in range(NT) for p in (0, 1)]
            pstate = {}

            def scores(r, p, j):
                # QK^T for both heads of the pair; the two 64-row matmuls run
                # concurrently on disjoint PE row halves.
                v = j - 4 * r
                off = 128 * v if v > 0 else 0   # q cols < off invalid
                if r > 0 and v == 0:
                    drain_until(f"K{r}o0")
                s2 = psum.tile([128, 1024], F32, tag="sc", bufs=2)
                nc.tensor.matmul(
                    s2[:, off:512],
                    KT[0:64, p, 128 * j : 128 * (j + 1)],
                    QT[0:64, p, 512 * r + off : 512 * (r + 1)],
                    start=True, stop=True,
                )
                nc.tensor.matmul(
                    s2[:, 512 + off : 1024],
                    KT[64:128, p, 128 * j : 128 * (j + 1)],
                    QT[64:128, p, 512 * r + off : 512 * (r + 1)],
                    start=True, stop=True,
                )
                pt2 = ppool.tile([128, 1024], MMDT, tag="pt")
                s2v = s2[:, :].rearrange("p (s q) -> p s q", s=2)
                pt2v = pt2[:, :].rearrange("p (s q) -> p s q", s=2)
                if v >= 0:      # diagonal block inside this q-range
                    nc.vector.tensor_add(
                        s2v[:, :, off : off + 128],
                        s2v[:, :, off : off + 128],
                        tri2,
                    )
                nc.scalar.activation(
                    pt2v[:, :, off:512], s2v[:, :, off:512],
                    EXPF, scale=SCALE,
                )
                return pt2, off

            def start_pair(r, p):
                # Two-pass PV: head A accumulates inline in the j-loop; head
                # B's whole PV chain is deferred as pumpable filler (it only
                # needs the persistent pt2 tiles). The LAST TWO pairs stay
                # fully inline so the single accB bank never has two
                # simultaneous users and the kernel tail doesn't grow.
                inline = pairs.index((r, p)) >= len(pairs) - 2
                ca = psum.tile([65, 512], F32, tag="accA", bufs=1,
                               name=f"ca{r}_{p}")
                # the inline cb is allocated lazily at its first write so the
                # accB slot rotation follows true first-write order (the
                # previous pair's deferred pass-B chain allocates first).
                pstate[(r, p)] = {"ca": ca, "cb": None, "pend": deque(),
                                  "bq": [], "inline": inline}

            def prefill_scores(r, p, j):
                pstate[(r, p)]["pend"].append(scores(r, p, j))

            def pv(r, p, j):
                st = pstate[(r, p)]
                pt2, off = st["pend"].popleft()
                v = j - 4 * r
                if r > 0 and 0 <= v < 4:
                    drain_until(f"V{r}t{v}")
                hA, hB = 2 * p, 2 * p + 1
                nj = 4 * r + 4
                nc.tensor.matmul(
                    st["ca"][:, off:512],
                    Vt[:, j, 65 * hA : 65 * hA + 65],
                    pt2[:, off:512],
                    start=(j == 0), stop=(j == nj - 1),
                )
                if st["inline"]:
                    if st["cb"] is None:
                        st["cb"] = psum.tile([65, 512], F32, tag="accB",
                                             bufs=1, name=f"cbi{r}_{p}")
                    nc.tensor.matmul(
                        st["cb"][:, off:512],
                        Vt[:, j, 65 * hB : 65 * hB + 65],
                        pt2[:, 512 + off : 1024],
                        start=(j == 0), stop=(j == nj - 1),
                    )
                else:
                    st["bq"].append((j, pt2, off))

            def normalize_head(r, p, head, acc):
                # stage, reciprocal of the ones-column denominator row,
                # gpsimd partition-broadcast, DVE multiply - one head only.
                sth = spool.tile([65, 512], F32, tag="st")
                nc.vector.tensor_copy(sth[:], acc[:])
                sr = spool.tile([1, 512], F32, tag="sw")
                nc.sync.dma_start(sr[0:1, :], sth[64:65, :])
                r1 = spool.tile([1, 512], F32, tag="r1")
                nc.vector.reciprocal_approx_fast(r1[:], sr[:])
                ra = spool.tile([64, 512], F32, tag="rc")
                nc.gpsimd.partition_broadcast(ra[:], r1[0:1, :])
                qs = slice(512 * r, 512 * (r + 1))
                rows = slice(0, 64) if head == 0 else slice(64, 128)
                nc.vector.tensor_mul(ctxT[rows, p, qs], sth[0:64, :], ra[:])

            def add_b_fillers(r, p):
                st = pstate[(r, p)]
                hB = 2 * p + 1
                nj = 4 * r + 4
                bst = {}
                def mk(j, pt2, off):
                    def go():
                        if j == 0:
                            bst["cb"] = psum.tile(
                                [65, 512], F32, tag="accB", bufs=1,
                                name=f"cbB{r}_{p}")
                        nc.tensor.matmul(
                            bst["cb"][:, off:512],
                            Vt[:, j, 65 * hB : 65 * hB + 65],
                            pt2[:, 512 + off : 1024],
                            start=(j == 0), stop=(j == nj - 1),
                        )
                    return (max(60, int(0.43 * (512 - off))), go)
                for (j, pt2, off) in st["bq"]:
                    fillers.append(mk(j, pt2, off))
                def normb():
                    normalize_head(r, p, 1, bst["cb"])
                fillers.append((0, normb))
                fillers.append((None, f"pB{pairs.index((r, p))}"))

            def normalize(r, p, is_last):
                st = pstate[(r, p)]
                ca, cb = st["ca"], st["cb"]
                if is_last:
                    # tail-critical pair: minimum-latency chain using a
                    # 1-deep PE row-broadcast of the denominator row
                    # (the PE is idle here anyway), then approx-fast
                    # reciprocal on the base-0 broadcast block. No staging
                    # copies: the multiplies read PSUM directly (the banks
                    # are not needed again - the kernel is ending).
                    dnA = spool.tile([128, 512], MMDT, tag="dn")
                    dnB = spool.tile([128, 512], MMDT, tag="dn")
                    nc.vector.tensor_copy(dnA[64:65, :], ca[64:65, :])
                    nc.vector.tensor_copy(dnB[64:65, :], cb[64:65, :])
                    bsA = psum.tile([64, 512], F32, tag="sc", bufs=2, name="bsA")
                    bsB = psum.tile([64, 512], F32, tag="sc", bufs=2, name="bsB")
                    nc.tensor.matmul(
                        bsA[:], ones128[64:65, :], dnA[64:65, :],
                        start=True, stop=True,
                    )
                    nc.tensor.matmul(
                        bsB[:], ones128[64:65, :], dnB[64:65, :],
                        start=True, stop=True,
                    )
                    ra = spool.tile([64, 512], F32, tag="rc")
                    rb = spool.tile([64, 512], F32, tag="rc")
                    nc.vector.reciprocal_approx_fast(ra[:], bsA[:])
                    nc.vector.reciprocal_approx_fast(rb[:], bsB[:])
                    qs = slice(512 * r, 512 * (r + 1))
                    nc.vector.tensor_mul(ctxT[0:64, p, qs], ca[0:64, :], ra[:])
                    nc.vector.tensor_mul(ctxT[64:128, p, qs], cb[0:64, :], rb[:])
                    return
                # stage accumulators to SBUF immediately (frees the PSUM
                # banks in ~1us); normalization then runs off the critical
                # path entirely from SBUF.
                stA = spool.tile([65, 512], F32, tag="st")
                stB = spool.tile([65, 512], F32, tag="st")
                nc.vector.tensor_copy(stA[:], ca[:])
                nc.vector.tensor_copy(stB[:], cb[:])
                if True:
                    # off the critical path: bounce the denominator rows
                    # to one 2-partition tile, one reciprocal for both
                    # heads, and broadcast on the gpsimd engine.
                    sr = spool.tile([1, 1024], F32, tag="sw")
                    nc.sync.dma_start(sr[0:1, 0:512], stA[64:65, :])
                    nc.sync.dma_start(sr[0:1, 512:1024], stB[64:65, :])
                    r12 = spool.tile([1, 1024], F32, tag="r1")
                    nc.vector.reciprocal_approx_fast(r12[:], sr[:])
                    ra = spool.tile([64, 512], F32, tag="rc")
                    rb = spool.tile([64, 512], F32, tag="rc")
                    nc.gpsimd.partition_broadcast(ra[:], r12[0:1, 0:512])
                    nc.gpsimd.partition_broadcast(rb[:], r12[0:1, 512:1024])
                qs = slice(512 * r, 512 * (r + 1))
                # keep the multiplies on DVE: gpsimd must stay on a single
                # microcode library (partition_broadcast) or it thrashes
                # LOAD_LIB/UNLOAD_LIB swaps with multi-us stalls.
                nc.vector.tensor_mul(ctxT[0:64, p, qs], stA[0:64, :], ra[:])
                nc.vector.tensor_mul(ctxT[64:128, p, qs], stB[0:64, :], rb[:])

            # ---------- startup ----------
            # DMA order is startup-critical: wq + first x chunks first so the
            # first Q-projection matmuls start as early as possible.
            wqv = wq.rearrange("p (c o) -> p c o", o=GW)
            nc.sync.dma_start(wq_sb[:], wqv)
            load_xt(0, split=True)
            nc.sync.dma_start(wk_sb[:], wk.rearrange("p (c o) -> p c o", o=GW))
            nc.sync.dma_start(tri[:], trid)
            nc.sync.dma_start(wv_sb[:], wv.rearrange("p (c o) -> p c o", o=GW))
            load_xt(1)
            nc.sync.dma_start(wo_sb[:], wo.rearrange("p (c o) -> p c o", o=D))

            # Only the pair-(0,0)-critical projections run before attention
            # starts; everything else becomes pumpable filler.
            for it in qk_items(0, wq_sb, QT, 0):
                it[1]()
            for it in qk_items(0, wk_sb, KT, 0):
                it[1]()

            # ---------- main pair loop ----------
            BUDGET_CAP = 2600
            PAIR_BONUS = 1100

            start_pair(0, 0)
            prefill_scores(0, 0, 0)
            prefill_scores(0, 0, 1)

            # pair-(0,1) projections BEFORE the V block: they only need
            # wq/wk/xt0 (landed), while V waits on the later wv DMA - V at
            # the queue head would stall everything behind it.
            for it in qk_items(0, wq_sb, QT, 1):
                it[1]()
            for it in qk_items(0, wk_sb, KT, 1):
                it[1]()
            markers_done.add("Q0o0")
            markers_done.add("K0o0")
            markers_done.add("a0")

            def finish_pair(r, p, idx):
                st = pstate[(r, p)]
                if st["inline"]:
                    normalize(r, p, is_last=(idx == len(pairs) - 1))
                else:
                    normalize_head(r, p, 0, st["ca"])
                    add_b_fillers(r, p)

            budget = 0
            for idx, (r, p) in enumerate(pairs):
                nj = 4 * r + 4
                if idx >= 2:
                    # bound pt2-pool pressure: pass B of the pair before last
                    # must be fully emitted before this pair's scores flood
                    # the pool with new exp outputs.
                    drain_until(f"pB{idx - 2}")
                if idx >= 1 and pstate[(r, p)]["inline"] \
                        and not pstate[pairs[idx - 1]]["inline"]:
                    # inline pair: the previous pair's pass-B chain must be
                    # fully emitted first, or its accB writes queue behind
                    # this pair's inline cb use -> PE-queue deadlock.
                    drain_until(f"pB{idx - 1}")
                if p == 0:
                    if r + 2 < NT:
                        load_xt(r + 2)
                    if r + 1 < NT:
                        add_a_fillers(r + 1)
                    if r >= 1:
                        add_c_fillers(r - 1)
                budget = min(budget + PAIR_BONUS, BUDGET_CAP)
                nxt = pairs[idx + 1] if idx + 1 < len(pairs) else None
                if r == 0:
                    # range-0 pairs (nj=4): the PVs wait on the late wv DMA;
                    # emit ALL scores and the next pair's prefill before any
                    # PV so the exp stream never queues behind V-gated work.
                    for j in (2, 3):
                        pstate[(r, p)]["pend"].append(scores(r, p, j))
                    if nxt is not None:
                        if nxt[0] != r:
                            drain_until(f"Q{nxt[0]}o0")
                        start_pair(*nxt)
                        prefill_scores(*nxt, 0)
                        prefill_scores(*nxt, 1)
                    if p == 0:
                        # V tiles emitted only now: wv lands late; anything
                        # queued behind a V matmul would stall the exp chain.
                        for tt in range(4):
                            for it in v_items(0, tt):
                                it[1]()
                            markers_done.add(f"V0t{tt}")
                    for j in range(nj):
                        pv(r, p, j)
                    finish_pair(r, p, idx)
                    budget = pump(min(budget + 2800, 4800))
                    continue
                for j in range(2, nj):
                    # scores(j) FIRST: it is on the exp-stream critical chain
                    # (slot freed by exp(j-2) -> scores(j) -> exp(j)); the PV
                    # pair's consumer is many js away, so it follows.
                    pstate[(r, p)]["pend"].append(scores(r, p, j))
                    pv(r, p, j - 2)
                    w = 512 - (128 * (j - 4 * r) if j - 4 * r > 0 else 0)
                    inc = (int(0.35 * w + 210) if pstate[(r, p)]["inline"]
                           else int(0.68 * w + 235))
                    budget = pump(min(budget + inc, 3600))
                # tail: drain last two PVs, prefilling the next pair's first
                # two score chunks in between so ACT never starves.
                if nxt is not None:
                    if nxt[0] != r:
                        drain_until(f"Q{nxt[0]}o0")
                    else:
                        # pair 1's prefill reads QT/KT o=1 of this range:
                        # force those chains out before emitting the reads.
                        drain_until(f"a{r}")
                    start_pair(*nxt)
                pv(r, p, nj - 2)
                if nxt is not None:
                    prefill_scores(*nxt, 0)
                pv(r, p, nj - 1)
                if nxt is not None:
                    prefill_scores(*nxt, 1)
                finish_pair(r, p, idx)
                # boundary window: the next pair's first two exps cover
                # ~2.2us of PE time - generous allowance drains a WO chain.
                budget = pump(min(budget + 2800, 4800))

            # Final drain: leftover fillers first, then the last range's WO
            # chains alternating pj/sc psum tags so FOUR slots keep the tail
            # pipeline deep. The first two chains' d=0 matmuls (pair-0 ctxT,
            # already normalized) are emitted up front so the PE stays busy
            # (and clocked up) through the final normalization latency.
            drain_until("__all__")
            chains = [
                wo_items(NT - 1, qq, o,
                         ptag=("pj" if (2 * qq + o) % 2 == 0 else "sc"))
                for qq in range(4) for o in range(2)
            ]
            for ch in chains[:2]:
                ch[0][1]()
            for ch in chains[:2]:
                ch[1][1]()
            for ch in chains[2:]:
                ch[0][1]()
                ch[1][1]()

    nc.compile()
    return nc


def _get_nc():
    if "nc" not in _CACHE:
        _CACHE["nc"] = _build()
    return _CACHE["nc"]


def kernel(x, Wq, Wk, Wv, Wo, bo):
    global LAST_RESULTS
    x = np.asarray(x, dtype=np.float32)
    Wq = np.asarray(Wq, dtype=np.float32)
    Wk = np.asarray(Wk, dtype=np.float32)
    Wv = np.asarray(Wv, dtype=np.float32)
    Wo = np.asarray(Wo, dtype=np.float32)
    bo = np.asarray(bo, dtype=np.float32)

    nc = _get_nc()
    # range-major layout: [NT*128, NC*512]; row 128*r+p holds chunks c=0..7
    # (512 tokens each, contiguous) of q-range r for feature-row p.

    def xarr(b):
        a = x[b].T.reshape(NC, 128, NT, 512).transpose(2, 1, 0, 3)
        return np.ascontiguousarray(a.reshape(NT * 128, NC * 512)).astype(NPDT)

    xTs = [xarr(b) for b in range(B)]

    def warr(w, cs):
        # [D, GW] slice -> [128, NC*GW]: partition p holds chunk-major rows
        s = w[:, cs].reshape(D // 128, 128, GW).transpose(1, 0, 2)
        return np.ascontiguousarray(s.reshape(128, -1)).astype(NPDT)

    def woarr(cs):
        # [GW, D] slice -> [128, 2*D]
        s = Wo[cs, :].reshape(GW // 128, 128, D).transpose(1, 0, 2)
        return np.ascontiguousarray(s.reshape(128, -1)).astype(NPDT)

    # causal mask block: keep (q - k >= 0) else -1e30  [partition=k, free=q]
    ktri = np.arange(128)
    tri_np = np.where(ktri[None, :] - ktri[:, None] >= 0, 0.0, NEG).astype(
        np.float32
    )

    in_maps = []
    for c in range(N_CORES):
        b, g = divmod(c, N_CORES // B)
        cs = slice(GW * g, GW * (g + 1))
        in_maps.append(
            {
                "xT": xTs[b],
                "wq": warr(Wq, cs),
                "wk": warr(Wk, cs),
                "wv": warr(Wv, cs),
                "wo": woarr(cs),
                "tri": tri_np,
            }
        )

    _maybe_install_trace_hook()
    res = bass_utils.run_bass_kernel_spmd(nc, in_maps, core_ids=list(range(N_CORES)))
    LAST_RESULTS = res

    out = np.zeros((B, S, D), dtype=np.float32)
    for c in range(N_CORES):
        out[c // (N_CORES // B)] += res.results[c]["out"].astype(np.float32)
    out += bo[None, None, :]
    return out


# revision 57
# speedup vs baseline: 1.0062x; 1.0062x over previous
"""Trainium2 Bass kernel for multi-head causal attention.

Problem: B=2, S=2048, D=1024, H=16 heads (head_dim=64), fp32.
  q,k,v = x@Wq, x@Wk, x@Wv  (per-head split)
  scores = q@k^T, causal mask, softmax(scores/sqrt(64))
  out = (attn@v concat) @ Wo + bo

Sharding (8 cores): core c -> batch b=c//4, head group g=c%4 (4 heads).
Each core computes its 4 heads' attention plus the partial output
projection (row-parallel Wo); host sums 4 partials per batch and adds bo.

Layout strategy (zero on-device transposes):
 - x^T passed host-transposed (feature-major).
 - Q^T,K^T produced feature-major: (head_dim x tokens), two heads stacked
   per 128-partition tile; scores^T computed per 64-partition row group.
 - Both heads' scores^T tiles (k x q) land in one 2-bank PSUM tile so the
   causal mask add + exp run as single wide instructions. The exp'd
   bf16 tile is directly the PV stationary operand. V is token-major with
   an appended ones-column so the PV matmul also emits the softmax
   denominators.

Scheduling: the attention inner loop is ACT-bound - each k-chunk's exp
(~1.1us) exceeds the PE work for that chunk (~0.65us scores+PV). All
projection/output-projection matmuls are therefore decomposed into
single-matmul work items and PUMPED into the per-chunk PE idle windows
with an ns-budget pacer, so the tensor engine streams continuously while
the scalar engine streams exps. Emission-order rules learned from traces:
 - scores(j) is emitted BEFORE pv(j-2) every iteration: the exp-stream
   critical chain is exp(j-2) ->[sc slot]-> scores(j) -> exp(j); the PV
   pair's consumer is many chunks away.
 - WO chains carry an inflated pump cost so they spread out ~1 chain per
   1.5 js: a dense WO burst makes chain n's psum-slot wait (freed by
   chain n-2's staging copy) back up into the in-order PE queue ahead of
   the attention stream.
 - Cross-pair score prefill between the two drain PVs keeps ACT fed
   across pair boundaries; range-0 pairs emit all scores before any PV
   because the PVs wait on the late wv DMA.
 - gpsimd runs ONLY partition_broadcast: mixing in other ops thrashes
   LOAD_LIB/UNLOAD_LIB microcode swaps with multi-us stalls; gpsimd also
   cannot touch PSUM, so all PSUM->SBUF staging is DVE (+ scalar for the
   o=0 WO half).
Softmax normalization (reciprocal of the ones-column denominators +
gpsimd partition broadcast + DVE multiply) runs off the critical path,
with a PE-row-broadcast fast path for the tail-critical final pair; the
final-range WO chains alternate into the idle scores psum tag for a
4-slot tail pipeline.

PSUM budget (8 banks): scores 2 tiles x 2 banks, PV accumulators 2 x 1,
projection/WO chains 2 x 1.
Matmul operands are bf16 (1 cycle/row PE rate); every accumulation and
the softmax normalization stay fp32 in PSUM.
"""

import sys

sys.path.insert(0, "/opt/trn_rl_repo")

from collections import deque

import ml_dtypes
import numpy as np

import concourse.bass as bass  # noqa: F401
import concourse.tile as tile
from concourse import bacc, bass_utils, mybir

F32 = mybir.dt.float32
MMDT = mybir.dt.bfloat16
NPDT = ml_dtypes.bfloat16
EXPF = mybir.ActivationFunctionType.Exp

B, S, D, H, HD = 2, 2048, 1024, 16, 64
N_CORES = 8
HPC = 4            # heads per core
GW = HPC * HD      # head-group width per core = 256
SCALE = 1.0 / np.sqrt(HD)
NEG = -1.0e30

NT = S // 512      # 4 q/t ranges of 512
NC = D // 128      # 8 contraction chunks for projections
NJ = S // 128      # 16 k-chunks

_CACHE = {}
LAST_RESULTS = None


def _maybe_install_trace_hook():
    """If BASS_TRACE is set, bass_utils needs antenv.axon_hooks (absent in
    this image). Install it from trn_boot when possible; otherwise disable
    tracing so the run still works."""
    import os

    if not os.environ.get("BASS_TRACE"):
        return
    try:
        import antenv.axon_hooks  # noqa: F401
        return
    except ImportError:
        pass
    try:
        import types

        from trn_agent_boot.trn_boot import _ntff_profile_via_ctypes

        hook = _ntff_profile_via_ctypes("/opt/axon/libaxon_pjrt.so")
        mod = types.ModuleType("antenv.axon_hooks")
        mod.get_axon_ntff_profile_hook = lambda: hook
        mod.set_axon_ntff_profile_hook = lambda h: None
        import antenv

        sys.modules["antenv.axon_hooks"] = mod
        antenv.axon_hooks = mod
    except Exception:
        os.environ["BASS_NEVER_TRACE"] = "1"


def _build():
    nc = bacc.Bacc("TRN2", target_bir_lowering=False, debug=False)

    # x is passed host-rearranged range-major: row 128*r+p holds the NC=8
    # contraction chunks for q-range r, each 512 tokens contiguous, so one
    # per-range DMA is 128 descriptors x 8KB.
    xT = nc.dram_tensor("xT", [(S // 512) * 128, (D // 128) * 512], MMDT,
                        kind="ExternalInput").ap()
    wq = nc.dram_tensor("wq", [128, D // 128 * GW], MMDT, kind="ExternalInput").ap()
    wk = nc.dram_tensor("wk", [128, D // 128 * GW], MMDT, kind="ExternalInput").ap()
    wv = nc.dram_tensor("wv", [128, D // 128 * GW], MMDT, kind="ExternalInput").ap()
    wo = nc.dram_tensor("wo", [128, GW // 128 * D], MMDT, kind="ExternalInput").ap()
    trid = nc.dram_tensor("tri", [128, 128], F32, kind="ExternalInput").ap()
    out = nc.dram_tensor("out", [S, D], MMDT, kind="ExternalOutput").ap()

    with tile.TileContext(nc) as tc, nc.allow_low_precision(reason="bf16 matmuls"):
        with (
            tc.tile_pool(name="const", bufs=1) as cpool,
            tc.tile_pool(name="xin", bufs=3) as xpool,
            tc.tile_pool(name="pt", bufs=32) as ppool,
            tc.tile_pool(name="small", bufs=4) as spool,
            tc.tile_pool(name="psum", bufs=1, space="PSUM") as psum,
        ):
            # ---- persistent tiles ----
            wq_sb = cpool.tile([128, NC, GW], MMDT)
            wk_sb = cpool.tile([128, NC, GW], MMDT)
            wv_sb = cpool.tile([128, NC, GW], MMDT)
            wo_sb = cpool.tile([128, 2, D], MMDT)

            QT = cpool.tile([128, 2, S], MMDT)   # [:, pair, t] feature-major
            KT = cpool.tile([128, 2, S], MMDT)
            Vt = cpool.tile([128, NJ, HPC * 65], MMDT)  # token-major + ones col
            ctxT = cpool.tile([128, 2, S], MMDT)

            # PE warmup: stream constant data through the tensor engine while
            # the first DMAs land, so the DVFS activity monitor ramps the PE
            # clock to max before real matmuls begin. One accumulation group
            # so no inter-instruction semaphores serialize it.
            warm_sb = cpool.tile([128, 512], MMDT, name="warm")
            nc.vector.memset(warm_sb[:], 0.125)
            warm_ps = psum.tile([128, 1024], F32, tag="sc", bufs=2)
            NWARM = 14
            for i in range(NWARM):
                nc.tensor.matmul(
                    warm_ps[:, 0:512], warm_sb[:, 0:128], warm_sb[:],
                    start=(i == 0), stop=(i == NWARM - 1),
                )
            # preload the Exp activation table during the DMA preamble so the
            # first real exp doesn't eat the lazy ACT_TABLE_LOAD.
            tbl = cpool.tile([1, 8], F32, name="tbl")
            nc.scalar.activation(tbl[:], warm_sb[0:1, 0:8], EXPF, scale=SCALE)

            # all-ones tile; row 64 is the stationary for the PE row-broadcast
            # of the softmax denominators (partition 64 -> partitions 0..63).
            ones128 = cpool.tile([128, 64], MMDT, name="ones128")
            nc.vector.memset(ones128[:], 1.0)

            # ones columns of V (col 64 of each 65-wide head slot)
            vt_ones = Vt[:, :, :].rearrange("p j (h u) -> p (j h) u", u=65)[:, :, 64:65]
            nc.vector.memset(vt_ones, 1.0)

            # triangular causal mask for the diagonal 128-block of scores^T:
            # keep (q - k >= 0) else -1e30   [partition = k, free = q]
            # (DMA emitted later, after the startup-critical wq/x transfers)
            tri = cpool.tile([128, 128], F32, name="tri")

            # broadcast view of tri over the two stacked heads (0-stride dim)
            tri_ap = tri[:]
            tri2 = bass.AP(
                tensor=tri_ap.tensor,
                offset=tri_ap.offset,
                ap=[list(tri_ap.ap[0]), [0, 2], list(tri_ap.ap[1])],
            )

            xts = {}

            def load_xt(r, split=False):
                xt = xpool.tile([128, NC, 512], MMDT, tag="xt")
                xv = xT[128 * r : 128 * (r + 1), :].rearrange(
                    "p (c t) -> p c t", t=512
                )
                if split:
                    # startup: land the first chunks earlier so the first
                    # projection matmuls can begin sooner.
                    nc.sync.dma_start(xt[:, 0:4, :], xv[:, 0:4, :])
                    nc.sync.dma_start(xt[:, 4:8, :], xv[:, 4:8, :])
                else:
                    nc.sync.dma_start(xt[:], xv)
                xts[r] = xt

            # ---------- filler work-item machinery ----------
            # Each item: (pe_cost_ns, fn). Markers gate force-drains so a
            # consumer can guarantee a producer chain has been emitted.
            fillers = deque()
            markers_done = set()

            def pump(budget):
                while fillers:
                    head = fillers[0]
                    if head[0] is None:
                        fillers.popleft()
                        markers_done.add(head[1])
                        continue
                    if head[0] > budget:
                        break
                    fillers.popleft()
                    head[1]()
                    budget -= head[0]
                return budget

            def drain_until(name):
                if name in markers_done:
                    return
                while fillers:
                    head = fillers.popleft()
                    if head[0] is None:
                        markers_done.add(head[1])
                        if head[1] == name:
                            return
                        continue
                    head[1]()

            def qk_items(r, w_sb, dst, o):
                st = {}
                def item(c):
                    def go():
                        if c == 0:
                            st["pm"] = psum.tile([128, 512], F32, tag="pj", bufs=2, name=f"pm{r}_{o}_{id(st)%97}")
                        nc.tensor.matmul(
                            st["pm"][:, :],
                            w_sb[:, c, 128 * o : 128 * (o + 1)],
                            xts[r][:, c, :],
                            start=(c == 0),
                            stop=(c == NC - 1),
                        )
                        if c == NC - 1:
                            # two half-copies: finer DVE granularity keeps
                            # the diagonal tri-adds from queuing behind a
                            # full-width copy.
                            nc.vector.tensor_copy(
                                dst[:, o, 512 * r : 512 * r + 256],
                                st["pm"][:, 0:256],
                            )
                            nc.vector.tensor_copy(
                                dst[:, o, 512 * r + 256 : 512 * (r + 1)],
                                st["pm"][:, 256:512],
                            )
                    return (216, go)
                return [item(c) for c in range(NC)]

            def v_items(r, tt):
                st = {}
                def item(c):
                    def go():
                        if c == 0:
                            st["pv"] = psum.tile([128, 512], F32, tag="pj", bufs=2, name=f"pvp{r}_{tt}")
                        nc.tensor.matmul(
                            st["pv"][:, 0:GW],
                            xts[r][:, c, 128 * tt : 128 * (tt + 1)],
                            wv_sb[:, c, :],
                            start=(c == 0),
                            stop=(c == NC - 1),
                        )
                        if c == NC - 1:
                            j = 4 * r + tt
                            vtv = Vt[:, j, :].rearrange(
                                "p (h u) -> p h u", u=65)
                            pvv = st["pv"][:, 0:GW].rearrange(
                                "p (h d) -> p h d", d=HD)
                            nc.vector.tensor_copy(
                                vtv[:, 0:2, 0:64], pvv[:, 0:2, :])
                            nc.vector.tensor_copy(
                                vtv[:, 2:4, 0:64], pvv[:, 2:4, :])
                    return (112, go)
                return [item(c) for c in range(NC)]

            def wo_items(r, qq, o, ptag="pj"):
                st = {}
                qt = 4 * r + qq
                def item(d):
                    def go():
                        if d == 0:
                            if ptag == "sc":
                                # tail chains borrow the (idle) scores psum
                                # tag for a deeper WO pipeline.
                                t = psum.tile([128, 1024], F32, tag="sc",
                                              bufs=2, name=f"posc{qt}_{o}")
                                st["po"] = t[:, 0:512]
                            else:
                                st["po"] = psum.tile(
                                    [128, 512], F32, tag="pj", bufs=2,
                                    name=f"po{qt}_{o}")[:, :]
                        nc.tensor.matmul(
                            st["po"],
                            ctxT[:, d, 128 * qt : 128 * (qt + 1)],
                            wo_sb[:, d, 512 * o : 512 * (o + 1)],
                            start=(d == 0), stop=(d == 1),
                        )
                        if d == 1:
                            # stage to SBUF (DMA cannot read PSUM), then DMA
                            # the bf16 partial out; host sums the partials.
                            # Copies alternate scalar/DVE to spread load.
                            ot = spool.tile(
                                [128, 512], MMDT, tag="ot", name=f"ot{qt}_{o}"
                            )
                            if o == 0:
                                nc.scalar.copy(ot[:], st["po"])
                            else:
                                nc.vector.tensor_copy(ot[:, 0:256], st["po"][:, 0:256])
                                nc.vector.tensor_copy(ot[:, 256:512], st["po"][:, 256:512])
                            nc.sync.dma_start(
                                out[128 * qt : 128 * (qt + 1),
                                    512 * o : 512 * (o + 1)],
                                ot[:],
                            )
                    # inflated pump cost: paces WO chains to ~one per 3-4
                    # j-iterations so a chain's psum-slot wait (freed by the
                    # previous chain's copy) never backs up into the in-order
                    # PE queue ahead of the attention stream.
                    return (450, go)
                return [item(d) for d in range(2)]

            def add_a_fillers(r):
                fillers.extend(qk_items(r, wq_sb, QT, 0))
                fillers.append((None, f"Q{r}o0"))
                fillers.extend(qk_items(r, wk_sb, KT, 0))
                fillers.append((None, f"K{r}o0"))
                for tt in range(4):
                    fillers.extend(v_items(r, tt))
                    fillers.append((None, f"V{r}t{tt}"))
                fillers.extend(qk_items(r, wq_sb, QT, 1))
                fillers.extend(qk_items(r, wk_sb, KT, 1))
                fillers.append((None, f"a{r}"))

            def add_c_fillers(r):
                for qq in range(4):
                    for o in range(2):
                        fillers.extend(wo_items(r, qq, o))

            # ---------- attention ----------
            pairs = [(r, p) for r in range(NT) for p in (0, 1)]
            pstate = {}

            def scores(r, p, j):
                # QK^T for both heads of the pair; the two 64-row matmuls run
                # concurrently on disjoint PE row halves.
                v = j - 4 * r
                off = 128 * v if v > 0 else 0   # q cols < off invalid
                if r > 0 and v == 0:
                    drain_until(f"K{r}o0")
                s2 = psum.tile([128, 1024], F32, tag="sc", bufs=2)
                nc.tensor.matmul(
                    s2[:, off:512],
                    KT[0:64, p, 128 * j : 128 * (j + 1)],
                    QT[0:64, p, 512 * r + off : 512 * (r + 1)],
                    start=True, stop=True,
                )
                nc.tensor.matmul(
                    s2[:, 512 + off : 1024],
                    KT[64:128, p, 128 * j : 128 * (j + 1)],
                    QT[64:128, p, 512 * r + off : 512 * (r + 1)],
                    start=True, stop=True,
                )
                pt2 = ppool.tile([128, 1024], MMDT, tag="pt")
                s2v = s2[:, :].rearrange("p (s q) -> p s q", s=2)
                pt2v = pt2[:, :].rearrange("p (s q) -> p s q", s=2)
                if v >= 0:      # diagonal block inside this q-range
                    nc.vector.tensor_add(
                        s2v[:, :, off : off + 128],
                        s2v[:, :, off : off + 128],
                        tri2,
                    )
                nc.scalar.activation(
                    pt2v[:, :, off:512], s2v[:, :, off:512],
                    EXPF, scale=SCALE,
                )
                return pt2, off

            def start_pair(r, p):
                # Two-pass PV: head A accumulates inline in the j-loop; head
                # B's whole PV chain is deferred as pumpable filler (it only
                # needs the persistent pt2 tiles). The LAST TWO pairs stay
                # fully inline so the single accB bank never has two
                # simultaneous users and the kernel tail doesn't grow.
                inline = pairs.index((r, p)) >= len(pairs) - 2
                ca = psum.tile([65, 512], F32, tag="accA", bufs=1,
                               name=f"ca{r}_{p}")
                # the inline cb is allocated lazily at its first write so the
                # accB slot rotation follows true first-write order (the
                # previous pair's deferred pass-B chain allocates first).
                pstate[(r, p)] = {"ca": ca, "cb": None, "pend": deque(),
                                  "bq": [], "inline": inline}

            def prefill_scores(r, p, j):
                pstate[(r, p)]["pend"].append(scores(r, p, j))

            def pv(r, p, j):
                st = pstate[(r, p)]
                pt2, off = st["pend"].popleft()
                v = j - 4 * r
                if r > 0 and 0 <= v < 4:
                    drain_until(f"V{r}t{v}")
                hA, hB = 2 * p, 2 * p + 1
                nj = 4 * r + 4
                nc.tensor.matmul(
                    st["ca"][:, off:512],
                    Vt[:, j, 65 * hA : 65 * hA + 65],
                    pt2[:, off:512],
                    start=(j == 0), stop=(j == nj - 1),
                )
                if st["inline"]:
                    if st["cb"] is None:
                        st["cb"] = psum.tile([65, 512], F32, tag="accB",
                                             bufs=1, name=f"cbi{r}_{p}")
                    nc.tensor.matmul(
                        st["cb"][:, off:512],
                        Vt[:, j, 65 * hB : 65 * hB + 65],
                        pt2[:, 512 + off : 1024],
                        start=(j == 0), stop=(j == nj - 1),
                    )
                else:
                    st["bq"].append((j, pt2, off))

            def normalize_head(r, p, head, acc):
                # stage, reciprocal of the ones-column denominator row,
                # gpsimd partition-broadcast, DVE multiply - one head only.
                sth = spool.tile([65, 512], F32, tag="st")
                nc.vector.tensor_copy(sth[:], acc[:])
                sr = spool.tile([1, 512], F32, tag="sw")
                nc.sync.dma_start(sr[0:1, :], sth[64:65, :])
                r1 = spool.tile([1, 512], F32, tag="r1")
                nc.vector.reciprocal_approx_fast(r1[:], sr[:])
                ra = spool.tile([64, 512], F32, tag="rc")
                nc.gpsimd.partition_broadcast(ra[:], r1[0:1, :])
                qs = slice(512 * r, 512 * (r + 1))
                rows = slice(0, 64) if head == 0 else slice(64, 128)
                nc.vector.tensor_mul(ctxT[rows, p, qs], sth[0:64, :], ra[:])

            def add_b_fillers(r, p):
                st = pstate[(r, p)]
                hB = 2 * p + 1
                nj = 4 * r + 4
                bst = {}
                def mk(j, pt2, off):
                    def go():
                        if j == 0:
                            bst["cb"] = psum.tile(
                                [65, 512], F32, tag="accB", bufs=1,
                                name=f"cbB{r}_{p}")
                        nc.tensor.matmul(
                            bst["cb"][:, off:512],
                            Vt[:, j, 65 * hB : 65 * hB + 65],
                            pt2[:, 512 + off : 1024],
                            start=(j == 0), stop=(j == nj - 1),
                        )
                    return (max(60, int(0.43 * (512 - off))), go)
                items = [mk(j, pt2, off) for (j, pt2, off) in st["bq"]]
                def normb():
                    normalize_head(r, p, 1, bst["cb"])
                items.append((0, normb))
                items.append((None, f"pB{pairs.index((r, p))}"))
                idxp = pairs.index((r, p))
                if idxp + 1 >= len(pairs) - 2:
                    # the NEXT pair is inline: this chain will be force-
                    # drained at its start. Clear older pass-B (so the accB
                    # allocation order stays first-write) and PREPEND, so the
                    # boundary pump drains most of it under the prefill-exp
                    # cover instead of as an uncovered block.
                    if idxp >= 1:
                        drain_until(f"pB{idxp - 1}")
                    fillers.extendleft(reversed(items))
                else:
                    fillers.extend(items)

            def normalize(r, p, is_last):
                st = pstate[(r, p)]
                ca, cb = st["ca"], st["cb"]
                if is_last:
                    # tail-critical pair: minimum-latency chain using a
                    # 1-deep PE row-broadcast of the denominator row
                    # (the PE is idle here anyway), then approx-fast
                    # reciprocal on the base-0 broadcast block. No staging
                    # copies: the multiplies read PSUM directly (the banks
                    # are not needed again - the kernel is ending).
                    dnA = spool.tile([128, 512], MMDT, tag="dn")
                    dnB = spool.tile([128, 512], MMDT, tag="dn")
                    nc.vector.tensor_copy(dnA[64:65, :], ca[64:65, :])
                    nc.vector.tensor_copy(dnB[64:65, :], cb[64:65, :])
                    bsA = psum.tile([64, 512], F32, tag="sc", bufs=2, name="bsA")
                    bsB = psum.tile([64, 512], F32, tag="sc", bufs=2, name="bsB")
                    nc.tensor.matmul(
                        bsA[:], ones128[64:65, :], dnA[64:65, :],
                        start=True, stop=True,
                    )
                    nc.tensor.matmul(
                        bsB[:], ones128[64:65, :], dnB[64:65, :],
                        start=True, stop=True,
                    )
                    ra = spool.tile([64, 512], F32, tag="rc")
                    rb = spool.tile([64, 512], F32, tag="rc")
                    nc.vector.reciprocal_approx_fast(ra[:], bsA[:])
                    nc.vector.reciprocal_approx_fast(rb[:], bsB[:])
                    qs = slice(512 * r, 512 * (r + 1))
                    nc.vector.tensor_mul(ctxT[0:64, p, qs], ca[0:64, :], ra[:])
                    nc.vector.tensor_mul(ctxT[64:128, p, qs], cb[0:64, :], rb[:])
                    return
                # stage accumulators to SBUF immediately (frees the PSUM
                # banks in ~1us); normalization then runs off the critical
                # path entirely from SBUF.
                stA = spool.tile([65, 512], F32, tag="st")
                stB = spool.tile([65, 512], F32, tag="st")
                nc.vector.tensor_copy(stA[:], ca[:])
                nc.vector.tensor_copy(stB[:], cb[:])
                if True:
                    # off the critical path: bounce the denominator rows
                    # to one 2-partition tile, one reciprocal for both
                    # heads, and broadcast on the gpsimd engine.
                    sr = spool.tile([1, 1024], F32, tag="sw")
                    nc.sync.dma_start(sr[0:1, 0:512], stA[64:65, :])
                    nc.sync.dma_start(sr[0:1, 512:1024], stB[64:65, :])
                    r12 = spool.tile([1, 1024], F32, tag="r1")
                    nc.vector.reciprocal_approx_fast(r12[:], sr[:])
                    ra = spool.tile([64, 512], F32, tag="rc")
                    rb = spool.tile([64, 512], F32, tag="rc")
                    nc.gpsimd.partition_broadcast(ra[:], r12[0:1, 0:512])
                    nc.gpsimd.partition_broadcast(rb[:], r12[0:1, 512:1024])
                qs = slice(512 * r, 512 * (r + 1))
                # keep the multiplies on DVE: gpsimd must stay on a single
                # microcode library (partition_broadcast) or it thrashes
                # LOAD_LIB/UNLOAD_LIB swaps with multi-us stalls.
                nc.vector.tensor_mul(ctxT[0:64, p, qs], stA[0:64, :], ra[:])
                nc.vector.tensor_mul(ctxT[64:128, p, qs], stB[0:64, :], rb[:])

            # ---------- startup ----------
            # DMA order is startup-critical: wq + first x chunks first so the
            # first Q-projection matmuls start as early as possible.
            wqv = wq.rearrange("p (c o) -> p c o", o=GW)
            nc.sync.dma_start(wq_sb[:], wqv)
            load_xt(0, split=True)
            nc.sync.dma_start(wk_sb[:], wk.rearrange("p (c o) -> p c o", o=GW))
            nc.sync.dma_start(tri[:], trid)
            nc.sync.dma_start(wv_sb[:], wv.rearrange("p (c o) -> p c o", o=GW))
            load_xt(1)
            nc.sync.dma_start(wo_sb[:], wo.rearrange("p (c o) -> p c o", o=D))

            # Only the pair-(0,0)-critical projections run before attention
            # starts; everything else becomes pumpable filler.
            for it in qk_items(0, wq_sb, QT, 0):
                it[1]()
            for it in qk_items(0, wk_sb, KT, 0):
                it[1]()

            # ---------- main pair loop ----------
            BUDGET_CAP = 2600
            PAIR_BONUS = 1100

            start_pair(0, 0)
            prefill_scores(0, 0, 0)
            prefill_scores(0, 0, 1)

            # pair-(0,1) projections BEFORE the V block: they only need
            # wq/wk/xt0 (landed), while V waits on the later wv DMA - V at
            # the queue head would stall everything behind it.
            for it in qk_items(0, wq_sb, QT, 1):
                it[1]()
            for it in qk_items(0, wk_sb, KT, 1):
                it[1]()
            markers_done.add("Q0o0")
            markers_done.add("K0o0")
            markers_done.add("a0")

            def finish_pair(r, p, idx):
                st = pstate[(r, p)]
                if st["inline"]:
                    normalize(r, p, is_last=(idx == len(pairs) - 1))
                else:
                    normalize_head(r, p, 0, st["ca"])
                    add_b_fillers(r, p)

            budget = 0
            for idx, (r, p) in enumerate(pairs):
                nj = 4 * r + 4
                if idx >= 2:
                    # bound pt2-pool pressure: pass B of the pair before last
                    # must be fully emitted before this pair's scores flood
                    # the pool with new exp outputs.
                    drain_until(f"pB{idx - 2}")
                if idx >= 1 and pstate[(r, p)]["inline"] \
                        and not pstate[pairs[idx - 1]]["inline"]:
                    # inline pair: the previous pair's pass-B chain must be
                    # fully emitted first, or its accB writes queue behind
                    # this pair's inline cb use -> PE-queue deadlock.
                    drain_until(f"pB{idx - 1}")
                if p == 0:
                    if r + 2 < NT:
                        load_xt(r + 2)
                    if r + 1 < NT:
                        add_a_fillers(r + 1)
                    if r >= 1:
                        add_c_fillers(r - 1)
                budget = min(budget + PAIR_BONUS, BUDGET_CAP)
                nxt = pairs[idx + 1] if idx + 1 < len(pairs) else None
                if r == 0:
                    # range-0 pairs (nj=4): the PVs wait on the late wv DMA;
                    # emit ALL scores and the next pair's prefill before any
                    # PV so the exp stream never queues behind V-gated work.
                    for j in (2, 3):
                        pstate[(r, p)]["pend"].append(scores(r, p, j))
                    if nxt is not None:
                        if nxt[0] != r:
                            drain_until(f"Q{nxt[0]}o0")
                        start_pair(*nxt)
                        prefill_scores(*nxt, 0)
                        prefill_scores(*nxt, 1)
                    if p == 0:
                        # V tiles emitted only now: wv lands late; anything
                        # queued behind a V matmul would stall the exp chain.
                        for tt in range(4):
                            for it in v_items(0, tt):
                                it[1]()
                            markers_done.add(f"V0t{tt}")
                    for j in range(nj):
                        pv(r, p, j)
                    finish_pair(r, p, idx)
                    budget = pump(min(budget + 2800, 4800))
                    continue
                for j in range(2, nj):
                    # scores(j) FIRST: it is on the exp-stream critical chain
                    # (slot freed by exp(j-2) -> scores(j) -> exp(j)); the PV
                    # pair's consumer is many js away, so it follows.
                    pstate[(r, p)]["pend"].append(scores(r, p, j))
                    pv(r, p, j - 2)
                    w = 512 - (128 * (j - 4 * r) if j - 4 * r > 0 else 0)
                    inc = (int(0.35 * w + 210) if pstate[(r, p)]["inline"]
                           else int(0.68 * w + 235))
                    budget = pump(min(budget + inc, 3600))
                # tail: drain last two PVs, prefilling the next pair's first
                # two score chunks in between so ACT never starves.
                if nxt is not None:
                    if nxt[0] != r:
                        drain_until(f"Q{nxt[0]}o0")
                    else:
                        # pair 1's prefill reads QT/KT o=1 of this range:
                        # force those chains out before emitting the reads.
                        drain_until(f"a{r}")
                    start_pair(*nxt)
                pv(r, p, nj - 2)
                if nxt is not None:
                    prefill_scores(*nxt, 0)
                pv(r, p, nj - 1)
                if nxt is not None:
                    prefill_scores(*nxt, 1)
                finish_pair(r, p, idx)
                # boundary window: the next pair's first two exps cover
                # ~2.2us of PE time - generous allowance drains a WO chain.
                budget = pump(min(budget + 2800, 4800))

            # Final drain: leftover fillers first, then the last range's WO
            # chains alternating pj/sc psum tags so FOUR slots keep the tail
            # pipeline deep. The first two chains' d=0 matmuls (pair-0 ctxT,
            # already normalized) are emitted up front so the PE stays busy
            # (and clocked up) through the final normalization latency.
            drain_until("__all__")
            chains = [
                wo_items(NT - 1, qq, o,
                         ptag=("pj" if (2 * qq + o) % 2 == 0 else "sc"))
                for qq in range(4) for o in range(2)
            ]
            for ch in chains[:2]:
                ch[0][1]()
            for ch in chains[:2]:
                ch[1][1]()
            for ch in chains[2:]:
                ch[0][1]()
                ch[1][1]()

    nc.compile()
    return nc


def _get_nc():
    if "nc" not in _CACHE:
        _CACHE["nc"] = _build()
    return _CACHE["nc"]


def kernel(x, Wq, Wk, Wv, Wo, bo):
    global LAST_RESULTS
    x = np.asarray(x, dtype=np.float32)
    Wq = np.asarray(Wq, dtype=np.float32)
    Wk = np.asarray(Wk, dtype=np.float32)
    Wv = np.asarray(Wv, dtype=np.float32)
    Wo = np.asarray(Wo, dtype=np.float32)
    bo = np.asarray(bo, dtype=np.float32)

    nc = _get_nc()
    # range-major layout: [NT*128, NC*512]; row 128*r+p holds chunks c=0..7
    # (512 tokens each, contiguous) of q-range r for feature-row p.

    def xarr(b):
        a = x[b].T.reshape(NC, 128, NT, 512).transpose(2, 1, 0, 3)
        return np.ascontiguousarray(a.reshape(NT * 128, NC * 512)).astype(NPDT)

    xTs = [xarr(b) for b in range(B)]

    def warr(w, cs):
        # [D, GW] slice -> [128, NC*GW]: partition p holds chunk-major rows
        s = w[:, cs].reshape(D // 128, 128, GW).transpose(1, 0, 2)
        return np.ascontiguousarray(s.reshape(128, -1)).astype(NPDT)

    def woarr(cs):
        # [GW, D] slice -> [128, 2*D]
        s = Wo[cs, :].reshape(GW // 128, 128, D).transpose(1, 0, 2)
        return np.ascontiguousarray(s.reshape(128, -1)).astype(NPDT)

    # causal mask block: keep (q - k >= 0) else -1e30  [partition=k, free=q]
    ktri = np.arange(128)
    tri_np = np.where(ktri[None, :] - ktri[:, None] >= 0, 0.0, NEG).astype(
        np.float32
    )

    in_maps = []
    for c in range(N_CORES):
        b, g = divmod(c, N_CORES // B)
        cs = slice(GW * g, GW * (g + 1))
        in_maps.append(
            {
                "xT": xTs[b],
                "wq": warr(Wq, cs),
                "wk": warr(Wk, cs),
                "wv": warr(Wv, cs),
                "wo": woarr(cs),
                "tri": tri_np,
            }
        )

    _maybe_install_trace_hook()
    res = bass_utils.run_bass_kernel_spmd(nc, in_maps, core_ids=list(range(N_CORES)))
    LAST_RESULTS = res

    out = np.zeros((B, S, D), dtype=np.float32)
    for c in range(N_CORES):
        out[c // (N_CORES // B)] += res.results[c]["out"].astype(np.float32)
    out += bo[None, None, :]
    return out


# revision 58
# speedup vs baseline: 1.0122x; 1.0060x over previous
"""Trainium2 Bass kernel for multi-head causal attention.

Problem: B=2, S=2048, D=1024, H=16 heads (head_dim=64), fp32.
  q,k,v = x@Wq, x@Wk, x@Wv  (per-head split)
  scores = q@k^T, causal mask, softmax(scores/sqrt(64))
  out = (attn@v concat) @ Wo + bo

Sharding (8 cores): core c -> batch b=c//4, head group g=c%4 (4 heads).
Each core computes its 4 heads' attention plus the partial output
projection (row-parallel Wo); host sums 4 partials per batch and adds bo.

Layout strategy (zero on-device transposes):
 - x^T passed host-transposed (feature-major).
 - Q^T,K^T produced feature-major: (head_dim x tokens), two heads stacked
   per 128-partition tile; scores^T computed per 64-partition row group.
 - Both heads' scores^T tiles (k x q) land in one 2-bank PSUM tile so the
   causal mask add + exp run as single wide instructions. The exp'd
   bf16 tile is directly the PV stationary operand. V is token-major with
   an appended ones-column so the PV matmul also emits the softmax
   denominators.

Scheduling: the attention inner loop is ACT-bound - each k-chunk's exp
(~1.1us) exceeds the PE work for that chunk (~0.65us scores+PV). All
projection/output-projection matmuls are therefore decomposed into
single-matmul work items and PUMPED into the per-chunk PE idle windows
with an ns-budget pacer, so the tensor engine streams continuously while
the scalar engine streams exps. Emission-order rules learned from traces:
 - scores(j) is emitted BEFORE pv(j-2) every iteration: the exp-stream
   critical chain is exp(j-2) ->[sc slot]-> scores(j) -> exp(j); the PV
   pair's consumer is many chunks away.
 - WO chains carry an inflated pump cost so they spread out ~1 chain per
   1.5 js: a dense WO burst makes chain n's psum-slot wait (freed by
   chain n-2's staging copy) back up into the in-order PE queue ahead of
   the attention stream.
 - Cross-pair score prefill between the two drain PVs keeps ACT fed
   across pair boundaries; range-0 pairs emit all scores before any PV
   because the PVs wait on the late wv DMA.
 - gpsimd runs ONLY partition_broadcast: mixing in other ops thrashes
   LOAD_LIB/UNLOAD_LIB microcode swaps with multi-us stalls; gpsimd also
   cannot touch PSUM, so all PSUM->SBUF staging is DVE (+ scalar for the
   o=0 WO half).
Softmax normalization (reciprocal of the ones-column denominators +
gpsimd partition broadcast + DVE multiply) runs off the critical path,
with a PE-row-broadcast fast path for the tail-critical final pair; the
final-range WO chains alternate into the idle scores psum tag for a
4-slot tail pipeline.

PSUM budget (8 banks): scores 2 tiles x 2 banks, PV accumulators 2 x 1,
projection/WO chains 2 x 1.
Matmul operands are bf16 (1 cycle/row PE rate); every accumulation and
the softmax normalization stay fp32 in PSUM.
"""

import sys

sys.path.insert(0, "/opt/trn_rl_repo")

from collections import deque

import ml_dtypes
import numpy as np

import concourse.bass as bass  # noqa: F401
import concourse.tile as tile
from concourse import bacc, bass_utils, mybir

F32 = mybir.dt.float32
MMDT = mybir.dt.bfloat16
NPDT = ml_dtypes.bfloat16
EXPF = mybir.ActivationFunctionType.Exp

B, S, D, H, HD = 2, 2048, 1024, 16, 64
N_CORES = 8
HPC = 4            # heads per core
GW = HPC * HD      # head-group width per core = 256
SCALE = 1.0 / np.sqrt(HD)
NEG = -1.0e30

NT = S // 512      # 4 q/t ranges of 512
NC = D // 128      # 8 contraction chunks for projections
NJ = S // 128      # 16 k-chunks

_CACHE = {}
LAST_RESULTS = None


def _maybe_install_trace_hook():
    """If BASS_TRACE is set, bass_utils needs antenv.axon_hooks (absent in
    this image). Install it from trn_boot when possible; otherwise disable
    tracing so the run still works."""
    import os

    if not os.environ.get("BASS_TRACE"):
        return
    try:
        import antenv.axon_hooks  # noqa: F401
        return
    except ImportError:
        pass
    try:
        import types

        from trn_agent_boot.trn_boot import _ntff_profile_via_ctypes

        hook = _ntff_profile_via_ctypes("/opt/axon/libaxon_pjrt.so")
        mod = types.ModuleType("antenv.axon_hooks")
        mod.get_axon_ntff_profile_hook = lambda: hook
        mod.set_axon_ntff_profile_hook = lambda h: None
        import antenv

        sys.modules["antenv.axon_hooks"] = mod
        antenv.axon_hooks = mod
    except Exception:
        os.environ["BASS_NEVER_TRACE"] = "1"


def _build():
    nc = bacc.Bacc("TRN2", target_bir_lowering=False, debug=False)

    # x is passed host-rearranged range-major: row 128*r+p holds the NC=8
    # contraction chunks for q-range r, each 512 tokens contiguous, so one
    # per-range DMA is 128 descriptors x 8KB.
    xT = nc.dram_tensor("xT", [(S // 512) * 128, (D // 128) * 512], MMDT,
                        kind="ExternalInput").ap()
    wq = nc.dram_tensor("wq", [128, D // 128 * GW], MMDT, kind="ExternalInput").ap()
    wk = nc.dram_tensor("wk", [128, D // 128 * GW], MMDT, kind="ExternalInput").ap()
    wv = nc.dram_tensor("wv", [128, D // 128 * GW], MMDT, kind="ExternalInput").ap()
    wo = nc.dram_tensor("wo", [128, GW // 128 * D], MMDT, kind="ExternalInput").ap()
    trid = nc.dram_tensor("tri", [128, 128], F32, kind="ExternalInput").ap()
    out = nc.dram_tensor("out", [S, D], MMDT, kind="ExternalOutput").ap()

    with tile.TileContext(nc) as tc, nc.allow_low_precision(reason="bf16 matmuls"):
        with (
            tc.tile_pool(name="const", bufs=1) as cpool,
            tc.tile_pool(name="xin", bufs=3) as xpool,
            tc.tile_pool(name="pt", bufs=32) as ppool,
            tc.tile_pool(name="small", bufs=4) as spool,
            tc.tile_pool(name="psum", bufs=1, space="PSUM") as psum,
        ):
            # ---- persistent tiles ----
            wq_sb = cpool.tile([128, NC, GW], MMDT)
            wk_sb = cpool.tile([128, NC, GW], MMDT)
            wv_sb = cpool.tile([128, NC, GW], MMDT)
            wo_sb = cpool.tile([128, 2, D], MMDT)

            QT = cpool.tile([128, 2, S], MMDT)   # [:, pair, t] feature-major
            KT = cpool.tile([128, 2, S], MMDT)
            Vt = cpool.tile([128, NJ, HPC * 65], MMDT)  # token-major + ones col
            ctxT = cpool.tile([128, 2, S], MMDT)

            # PE warmup: stream constant data through the tensor engine while
            # the first DMAs land, so the DVFS activity monitor ramps the PE
            # clock to max before real matmuls begin. One accumulation group
            # so no inter-instruction semaphores serialize it.
            warm_sb = cpool.tile([128, 512], MMDT, name="warm")
            nc.vector.memset(warm_sb[:], 0.125)
            warm_ps = psum.tile([128, 1024], F32, tag="sc", bufs=2)
            NWARM = 14
            for i in range(NWARM):
                nc.tensor.matmul(
                    warm_ps[:, 0:512], warm_sb[:, 0:128], warm_sb[:],
                    start=(i == 0), stop=(i == NWARM - 1),
                )
            # preload the Exp activation table during the DMA preamble so the
            # first real exp doesn't eat the lazy ACT_TABLE_LOAD.
            tbl = cpool.tile([1, 8], F32, name="tbl")
            nc.scalar.activation(tbl[:], warm_sb[0:1, 0:8], EXPF, scale=SCALE)

            # all-ones tile; row 64 is the stationary for the PE row-broadcast
            # of the softmax denominators (partition 64 -> partitions 0..63).
            ones128 = cpool.tile([128, 64], MMDT, name="ones128")
            nc.vector.memset(ones128[:], 1.0)

            # ones columns of V (col 64 of each 65-wide head slot)
            vt_ones = Vt[:, :, :].rearrange("p j (h u) -> p (j h) u", u=65)[:, :, 64:65]
            nc.vector.memset(vt_ones, 1.0)

            # triangular causal mask for the diagonal 128-block of scores^T:
            # keep (q - k >= 0) else -1e30   [partition = k, free = q]
            # (DMA emitted later, after the startup-critical wq/x transfers)
            tri = cpool.tile([128, 128], F32, name="tri")

            # broadcast view of tri over the two stacked heads (0-stride dim)
            tri_ap = tri[:]
            tri2 = bass.AP(
                tensor=tri_ap.tensor,
                offset=tri_ap.offset,
                ap=[list(tri_ap.ap[0]), [0, 2], list(tri_ap.ap[1])],
            )

            xts = {}

            def load_xt(r, split=False):
                xt = xpool.tile([128, NC, 512], MMDT, tag="xt")
                xv = xT[128 * r : 128 * (r + 1), :].rearrange(
                    "p (c t) -> p c t", t=512
                )
                if split:
                    # startup: land the first chunks earlier so the first
                    # projection matmuls can begin sooner.
                    nc.sync.dma_start(xt[:, 0:4, :], xv[:, 0:4, :])
                    nc.sync.dma_start(xt[:, 4:8, :], xv[:, 4:8, :])
                else:
                    nc.sync.dma_start(xt[:], xv)
                xts[r] = xt

            # ---------- filler work-item machinery ----------
            # Each item: (pe_cost_ns, fn). Markers gate force-drains so a
            # consumer can guarantee a producer chain has been emitted.
            fillers = deque()
            markers_done = set()

            def pump(budget):
                while fillers:
                    head = fillers[0]
                    if head[0] is None:
                        fillers.popleft()
                        markers_done.add(head[1])
                        continue
                    if head[0] > budget:
                        break
                    fillers.popleft()
                    head[1]()
                    budget -= head[0]
                return budget

            def drain_until(name):
                if name in markers_done:
                    return
                while fillers:
                    head = fillers.popleft()
                    if head[0] is None:
                        markers_done.add(head[1])
                        if head[1] == name:
                            return
                        continue
                    head[1]()

            def qk_items(r, w_sb, dst, o):
                st = {}
                def item(c):
                    def go():
                        if c == 0:
                            st["pm"] = psum.tile([128, 512], F32, tag="pj", bufs=2, name=f"pm{r}_{o}_{id(st)%97}")
                        nc.tensor.matmul(
                            st["pm"][:, :],
                            w_sb[:, c, 128 * o : 128 * (o + 1)],
                            xts[r][:, c, :],
                            start=(c == 0),
                            stop=(c == NC - 1),
                        )
                        if c == NC - 1:
                            # two half-copies: finer DVE granularity keeps
                            # the diagonal tri-adds from queuing behind a
                            # full-width copy.
                            nc.vector.tensor_copy(
                                dst[:, o, 512 * r : 512 * r + 256],
                                st["pm"][:, 0:256],
                            )
                            nc.vector.tensor_copy(
                                dst[:, o, 512 * r + 256 : 512 * (r + 1)],
                                st["pm"][:, 256:512],
                            )
                    return (216, go)
                return [item(c) for c in range(NC)]

            def v_items(r, tt):
                st = {}
                def item(c):
                    def go():
                        if c == 0:
                            st["pv"] = psum.tile([128, 512], F32, tag="pj", bufs=2, name=f"pvp{r}_{tt}")
                        nc.tensor.matmul(
                            st["pv"][:, 0:GW],
                            xts[r][:, c, 128 * tt : 128 * (tt + 1)],
                            wv_sb[:, c, :],
                            start=(c == 0),
                            stop=(c == NC - 1),
                        )
                        if c == NC - 1:
                            j = 4 * r + tt
                            vtv = Vt[:, j, :].rearrange(
                                "p (h u) -> p h u", u=65)
                            pvv = st["pv"][:, 0:GW].rearrange(
                                "p (h d) -> p h d", d=HD)
                            nc.vector.tensor_copy(
                                vtv[:, 0:2, 0:64], pvv[:, 0:2, :])
                            nc.vector.tensor_copy(
                                vtv[:, 2:4, 0:64], pvv[:, 2:4, :])
                    return (112, go)
                return [item(c) for c in range(NC)]

            def wo_items(r, qq, o, ptag="pj"):
                st = {}
                qt = 4 * r + qq
                def item(d):
                    def go():
                        if d == 0:
                            if ptag == "sc":
                                # tail chains borrow the (idle) scores psum
                                # tag for a deeper WO pipeline.
                                t = psum.tile([128, 1024], F32, tag="sc",
                                              bufs=2, name=f"posc{qt}_{o}")
                                st["po"] = t[:, 0:512]
                            else:
                                st["po"] = psum.tile(
                                    [128, 512], F32, tag="pj", bufs=2,
                                    name=f"po{qt}_{o}")[:, :]
                        nc.tensor.matmul(
                            st["po"],
                            ctxT[:, d, 128 * qt : 128 * (qt + 1)],
                            wo_sb[:, d, 512 * o : 512 * (o + 1)],
                            start=(d == 0), stop=(d == 1),
                        )
                        if d == 1:
                            # stage to SBUF (DMA cannot read PSUM), then DMA
                            # the bf16 partial out; host sums the partials.
                            # Copies alternate scalar/DVE to spread load.
                            ot = spool.tile(
                                [128, 512], MMDT, tag="ot", name=f"ot{qt}_{o}"
                            )
                            if o == 0:
                                nc.scalar.copy(ot[:], st["po"])
                            else:
                                nc.vector.tensor_copy(ot[:, 0:256], st["po"][:, 0:256])
                                nc.vector.tensor_copy(ot[:, 256:512], st["po"][:, 256:512])
                            nc.sync.dma_start(
                                out[128 * qt : 128 * (qt + 1),
                                    512 * o : 512 * (o + 1)],
                                ot[:],
                            )
                    # inflated pump cost: paces WO chains to ~one per 3-4
                    # j-iterations so a chain's psum-slot wait (freed by the
                    # previous chain's copy) never backs up into the in-order
                    # PE queue ahead of the attention stream.
                    return (450, go)
                return [item(d) for d in range(2)]

            def add_a_fillers(r):
                fillers.extend(qk_items(r, wq_sb, QT, 0))
                fillers.append((None, f"Q{r}o0"))
                fillers.extend(qk_items(r, wk_sb, KT, 0))
                fillers.append((None, f"K{r}o0"))
                for tt in range(4):
                    fillers.extend(v_items(r, tt))
                    fillers.append((None, f"V{r}t{tt}"))
                fillers.extend(qk_items(r, wq_sb, QT, 1))
                fillers.extend(qk_items(r, wk_sb, KT, 1))
                fillers.append((None, f"a{r}"))

            def add_c_fillers(r):
                for qq in range(4):
                    for o in range(2):
                        fillers.extend(wo_items(r, qq, o))

            # ---------- attention ----------
            pairs = [(r, p) for r in range(NT) for p in (0, 1)]
            pstate = {}

            def scores(r, p, j):
                # QK^T for both heads of the pair; the two 64-row matmuls run
                # concurrently on disjoint PE row halves.
                v = j - 4 * r
                off = 128 * v if v > 0 else 0   # q cols < off invalid
                if r > 0 and v == 0:
                    drain_until(f"K{r}o0")
                s2 = psum.tile([128, 1024], F32, tag="sc", bufs=2)
                nc.tensor.matmul(
                    s2[:, off:512],
                    KT[0:64, p, 128 * j : 128 * (j + 1)],
                    QT[0:64, p, 512 * r + off : 512 * (r + 1)],
                    start=True, stop=True,
                )
                nc.tensor.matmul(
                    s2[:, 512 + off : 1024],
                    KT[64:128, p, 128 * j : 128 * (j + 1)],
                    QT[64:128, p, 512 * r + off : 512 * (r + 1)],
                    start=True, stop=True,
                )
                pt2 = ppool.tile([128, 1024], MMDT, tag="pt")
                s2v = s2[:, :].rearrange("p (s q) -> p s q", s=2)
                pt2v = pt2[:, :].rearrange("p (s q) -> p s q", s=2)
                if v >= 0:      # diagonal block inside this q-range
                    nc.vector.tensor_add(
                        s2v[:, :, off : off + 128],
                        s2v[:, :, off : off + 128],
                        tri2,
                    )
                nc.scalar.activation(
                    pt2v[:, :, off:512], s2v[:, :, off:512],
                    EXPF, scale=SCALE,
                )
                return pt2, off

            def start_pair(r, p):
                # Two-pass PV: head A accumulates inline in the j-loop; head
                # B's whole PV chain is deferred as pumpable filler (it only
                # needs the persistent pt2 tiles). The LAST TWO pairs stay
                # fully inline so the single accB bank never has two
                # simultaneous users and the kernel tail doesn't grow.
                inline = pairs.index((r, p)) >= len(pairs) - 2
                ca = psum.tile([65, 512], F32, tag="accA", bufs=1,
                               name=f"ca{r}_{p}")
                # the inline cb is allocated lazily at its first write so the
                # accB slot rotation follows true first-write order (the
                # previous pair's deferred pass-B chain allocates first).
                pstate[(r, p)] = {"ca": ca, "cb": None, "pend": deque(),
                                  "bq": [], "inline": inline}

            def prefill_scores(r, p, j):
                pstate[(r, p)]["pend"].append(scores(r, p, j))

            def pv(r, p, j):
                st = pstate[(r, p)]
                pt2, off = st["pend"].popleft()
                v = j - 4 * r
                if r > 0 and 0 <= v < 4:
                    drain_until(f"V{r}t{v}")
                hA, hB = 2 * p, 2 * p + 1
                nj = 4 * r + 4
                nc.tensor.matmul(
                    st["ca"][:, off:512],
                    Vt[:, j, 65 * hA : 65 * hA + 65],
                    pt2[:, off:512],
                    start=(j == 0), stop=(j == nj - 1),
                )
                if st["inline"]:
                    if st["cb"] is None:
                        st["cb"] = psum.tile([65, 512], F32, tag="accB",
                                             bufs=1, name=f"cbi{r}_{p}")
                    nc.tensor.matmul(
                        st["cb"][:, off:512],
                        Vt[:, j, 65 * hB : 65 * hB + 65],
                        pt2[:, 512 + off : 1024],
                        start=(j == 0), stop=(j == nj - 1),
                    )
                else:
                    st["bq"].append((j, pt2, off))

            def normalize_head(r, p, head, acc):
                # stage, reciprocal of the ones-column denominator row,
                # gpsimd partition-broadcast, DVE multiply - one head only.
                sth = spool.tile([65, 512], F32, tag="st")
                nc.vector.tensor_copy(sth[:], acc[:])
                sr = spool.tile([1, 512], F32, tag="sw")
                nc.sync.dma_start(sr[0:1, :], sth[64:65, :])
                r1 = spool.tile([1, 512], F32, tag="r1")
                nc.vector.reciprocal_approx_fast(r1[:], sr[:])
                ra = spool.tile([64, 512], F32, tag="rc")
                nc.gpsimd.partition_broadcast(ra[:], r1[0:1, :])
                qs = slice(512 * r, 512 * (r + 1))
                rows = slice(0, 64) if head == 0 else slice(64, 128)
                nc.vector.tensor_mul(ctxT[rows, p, qs], sth[0:64, :], ra[:])

            def add_b_fillers(r, p):
                st = pstate[(r, p)]
                hB = 2 * p + 1
                nj = 4 * r + 4
                bst = {}
                def mk(j, pt2, off):
                    def go():
                        if j == 0:
                            bst["cb"] = psum.tile(
                                [65, 512], F32, tag="accB", bufs=1,
                                name=f"cbB{r}_{p}")
                        nc.tensor.matmul(
                            bst["cb"][:, off:512],
                            Vt[:, j, 65 * hB : 65 * hB + 65],
                            pt2[:, 512 + off : 1024],
                            start=(j == 0), stop=(j == nj - 1),
                        )
                    return (max(60, int(0.43 * (512 - off))), go)
                items = [mk(j, pt2, off) for (j, pt2, off) in st["bq"]]
                def normb():
                    normalize_head(r, p, 1, bst["cb"])
                items.append((0, normb))
                items.append((None, f"pB{pairs.index((r, p))}"))
                idxp = pairs.index((r, p))
                if idxp + 1 >= len(pairs) - 2:
                    # the NEXT pair is inline: this chain will be force-
                    # drained at its start. Clear older pass-B (so the accB
                    # allocation order stays first-write) and PREPEND, so the
                    # boundary pump drains most of it under the prefill-exp
                    # cover instead of as an uncovered block.
                    if idxp >= 1:
                        drain_until(f"pB{idxp - 1}")
                    fillers.extendleft(reversed(items))
                else:
                    fillers.extend(items)

            def normalize(r, p, is_last):
                st = pstate[(r, p)]
                ca, cb = st["ca"], st["cb"]
                if is_last:
                    # tail-critical pair: minimum-latency chain using a
                    # 1-deep PE row-broadcast of the denominator row
                    # (the PE is idle here anyway), then approx-fast
                    # reciprocal on the base-0 broadcast block. No staging
                    # copies: the multiplies read PSUM directly (the banks
                    # are not needed again - the kernel is ending).
                    dnA = spool.tile([128, 512], MMDT, tag="dn")
                    dnB = spool.tile([128, 512], MMDT, tag="dn")
                    nc.vector.tensor_copy(dnA[64:65, :], ca[64:65, :])
                    nc.vector.tensor_copy(dnB[64:65, :], cb[64:65, :])
                    bsA = psum.tile([64, 512], F32, tag="sc", bufs=2, name="bsA")
                    bsB = psum.tile([64, 512], F32, tag="sc", bufs=2, name="bsB")
                    nc.tensor.matmul(
                        bsA[:], ones128[64:65, :], dnA[64:65, :],
                        start=True, stop=True,
                    )
                    nc.tensor.matmul(
                        bsB[:], ones128[64:65, :], dnB[64:65, :],
                        start=True, stop=True,
                    )
                    ra = spool.tile([64, 512], F32, tag="rc")
                    rb = spool.tile([64, 512], F32, tag="rc")
                    nc.vector.reciprocal_approx_fast(ra[:], bsA[:])
                    nc.vector.reciprocal_approx_fast(rb[:], bsB[:])
                    qs = slice(512 * r, 512 * (r + 1))
                    nc.vector.tensor_mul(ctxT[0:64, p, qs], ca[0:64, :], ra[:])
                    nc.vector.tensor_mul(ctxT[64:128, p, qs], cb[0:64, :], rb[:])
                    return
                # stage accumulators to SBUF immediately (frees the PSUM
                # banks in ~1us); normalization then runs off the critical
                # path entirely from SBUF.
                stA = spool.tile([65, 512], F32, tag="st")
                stB = spool.tile([65, 512], F32, tag="st")
                nc.vector.tensor_copy(stA[:], ca[:])
                nc.vector.tensor_copy(stB[:], cb[:])
                if True:
                    # off the critical path: bounce the denominator rows
                    # to one 2-partition tile, one reciprocal for both
                    # heads, and broadcast on the gpsimd engine.
                    sr = spool.tile([1, 1024], F32, tag="sw")
                    nc.sync.dma_start(sr[0:1, 0:512], stA[64:65, :])
                    nc.sync.dma_start(sr[0:1, 512:1024], stB[64:65, :])
                    r12 = spool.tile([1, 1024], F32, tag="r1")
                    nc.vector.reciprocal_approx_fast(r12[:], sr[:])
                    ra = spool.tile([64, 512], F32, tag="rc")
                    rb = spool.tile([64, 512], F32, tag="rc")
                    nc.gpsimd.partition_broadcast(ra[:], r12[0:1, 0:512])
                    nc.gpsimd.partition_broadcast(rb[:], r12[0:1, 512:1024])
                qs = slice(512 * r, 512 * (r + 1))
                # keep the multiplies on DVE: gpsimd must stay on a single
                # microcode library (partition_broadcast) or it thrashes
                # LOAD_LIB/UNLOAD_LIB swaps with multi-us stalls.
                nc.vector.tensor_mul(ctxT[0:64, p, qs], stA[0:64, :], ra[:])
                nc.vector.tensor_mul(ctxT[64:128, p, qs], stB[0:64, :], rb[:])

            # ---------- startup ----------
            # DMA order is startup-critical: wq + first x chunks first so the
            # first Q-projection matmuls start as early as possible.
            wqv = wq.rearrange("p (c o) -> p c o", o=GW)
            nc.sync.dma_start(wq_sb[:], wqv)
            load_xt(0, split=True)
            nc.sync.dma_start(wk_sb[:], wk.rearrange("p (c o) -> p c o", o=GW))
            nc.sync.dma_start(tri[:], trid)
            nc.sync.dma_start(wv_sb[:], wv.rearrange("p (c o) -> p c o", o=GW))
            load_xt(1)
            nc.sync.dma_start(wo_sb[:], wo.rearrange("p (c o) -> p c o", o=D))

            # Only the pair-(0,0)-critical projections run before attention
            # starts; everything else becomes pumpable filler.
            for it in qk_items(0, wq_sb, QT, 0):
                it[1]()
            for it in qk_items(0, wk_sb, KT, 0):
                it[1]()

            # ---------- main pair loop ----------
            BUDGET_CAP = 2600
            PAIR_BONUS = 1100

            start_pair(0, 0)
            prefill_scores(0, 0, 0)
            prefill_scores(0, 0, 1)

            # pair-(0,1) projections BEFORE the V block: they only need
            # wq/wk/xt0 (landed), while V waits on the later wv DMA - V at
            # the queue head would stall everything behind it.
            for it in qk_items(0, wq_sb, QT, 1):
                it[1]()
            for it in qk_items(0, wk_sb, KT, 1):
                it[1]()
            markers_done.add("Q0o0")
            markers_done.add("K0o0")
            markers_done.add("a0")

            def finish_pair(r, p, idx):
                st = pstate[(r, p)]
                if st["inline"]:
                    normalize(r, p, is_last=(idx == len(pairs) - 1))
                else:
                    normalize_head(r, p, 0, st["ca"])
                    add_b_fillers(r, p)

            budget = 0
            for idx, (r, p) in enumerate(pairs):
                nj = 4 * r + 4
                if idx >= 2:
                    # bound pt2-pool pressure: pass B of the pair before last
                    # must be fully emitted before this pair's scores flood
                    # the pool with new exp outputs.
                    drain_until(f"pB{idx - 2}")
                if idx >= 1 and pstate[(r, p)]["inline"] \
                        and not pstate[pairs[idx - 1]]["inline"]:
                    # inline pair: the previous pair's pass-B chain must be
                    # fully emitted first, or its accB writes queue behind
                    # this pair's inline cb use -> PE-queue deadlock.
                    drain_until(f"pB{idx - 1}")
                if p == 0:
                    if r + 2 < NT:
                        load_xt(r + 2)
                    if r + 1 < NT:
                        add_a_fillers(r + 1)
                    # WO deferred by two ranges: range 2's pump capacity is
                    # fully claimed by the a3 projection chains; the two-pass
                    # j-loops of range 3 have spare capacity for the WO load.
                    if r >= 2:
                        add_c_fillers(r - 2)
                    if r == NT - 1:
                        add_c_fillers(r - 1)
                budget = min(budget + PAIR_BONUS, BUDGET_CAP)
                nxt = pairs[idx + 1] if idx + 1 < len(pairs) else None
                if r == 0:
                    # range-0 pairs (nj=4): the PVs wait on the late wv DMA;
                    # emit ALL scores and the next pair's prefill before any
                    # PV so the exp stream never queues behind V-gated work.
                    for j in (2, 3):
                        pstate[(r, p)]["pend"].append(scores(r, p, j))
                    if nxt is not None:
                        if nxt[0] != r:
                            drain_until(f"Q{nxt[0]}o0")
                        start_pair(*nxt)
                        prefill_scores(*nxt, 0)
                        prefill_scores(*nxt, 1)
                    if p == 0:
                        # V tiles emitted only now: wv lands late; anything
                        # queued behind a V matmul would stall the exp chain.
                        for tt in range(4):
                            for it in v_items(0, tt):
                                it[1]()
                            markers_done.add(f"V0t{tt}")
                    for j in range(nj):
                        pv(r, p, j)
                    finish_pair(r, p, idx)
                    budget = pump(min(budget + 2800, 4800))
                    continue
                for j in range(2, nj):
                    # scores(j) FIRST: it is on the exp-stream critical chain
                    # (slot freed by exp(j-2) -> scores(j) -> exp(j)); the PV
                    # pair's consumer is many js away, so it follows.
                    pstate[(r, p)]["pend"].append(scores(r, p, j))
                    pv(r, p, j - 2)
                    w = 512 - (128 * (j - 4 * r) if j - 4 * r > 0 else 0)
                    inc = (int(0.35 * w + 210) if pstate[(r, p)]["inline"]
                           else int(0.68 * w + 235))
                    budget = pump(min(budget + inc, 3600))
                # tail: drain last two PVs, prefilling the next pair's first
                # two score chunks in between so ACT never starves.
                if nxt is not None:
                    if nxt[0] != r:
                        drain_until(f"Q{nxt[0]}o0")
                    else:
                        # pair 1's prefill reads QT/KT o=1 of this range:
                        # force those chains out before emitting the reads.
                        drain_until(f"a{r}")
                    start_pair(*nxt)
                pv(r, p, nj - 2)
                if nxt is not None:
                    prefill_scores(*nxt, 0)
                pv(r, p, nj - 1)
                if nxt is not None:
                    prefill_scores(*nxt, 1)
                finish_pair(r, p, idx)
                # boundary window: the next pair's first two exps cover
                # ~2.2us of PE time - generous allowance drains a WO chain.
                budget = pump(min(budget + 2800, 4800))

            # Final drain: leftover fillers first, then the last range's WO
            # chains alternating pj/sc psum tags so FOUR slots keep the tail
            # pipeline deep. The first two chains' d=0 matmuls (pair-0 ctxT,
            # already normalized) are emitted up front so the PE stays busy
            # (and clocked up) through the final normalization latency.
            drain_until("__all__")
            chains = [
                wo_items(NT - 1, qq, o,
                         ptag=("pj" if (2 * qq + o) % 2 == 0 else "sc"))
                for qq in range(4) for o in range(2)
            ]
            for ch in chains[:2]:
                ch[0][1]()
            for ch in chains[:2]:
                ch[1][1]()
            for ch in chains[2:]:
                ch[0][1]()
                ch[1][1]()

    nc.compile()
    return nc


def _get_nc():
    if "nc" not in _CACHE:
        _CACHE["nc"] = _build()
    return _CACHE["nc"]


def kernel(x, Wq, Wk, Wv, Wo, bo):
    global LAST_RESULTS
    x = np.asarray(x, dtype=np.float32)
    Wq = np.asarray(Wq, dtype=np.float32)
    Wk = np.asarray(Wk, dtype=np.float32)
    Wv = np.asarray(Wv, dtype=np.float32)
    Wo = np.asarray(Wo, dtype=np.float32)
    bo = np.asarray(bo, dtype=np.float32)

    nc = _get_nc()
    # range-major layout: [NT*128, NC*512]; row 128*r+p holds chunks c=0..7
    # (512 tokens each, contiguous) of q-range r for feature-row p.

    def xarr(b):
        a = x[b].T.reshape(NC, 128, NT, 512).transpose(2, 1, 0, 3)
        return np.ascontiguousarray(a.reshape(NT * 128, NC * 512)).astype(NPDT)

    xTs = [xarr(b) for b in range(B)]

    def warr(w, cs):
        # [D, GW] slice -> [128, NC*GW]: partition p holds chunk-major rows
        s = w[:, cs].reshape(D // 128, 128, GW).transpose(1, 0, 2)
        return np.ascontiguousarray(s.reshape(128, -1)).astype(NPDT)

    def woarr(cs):
        # [GW, D] slice -> [128, 2*D]
        s = Wo[cs, :].reshape(GW // 128, 128, D).transpose(1, 0, 2)
        return np.ascontiguousarray(s.reshape(128, -1)).astype(NPDT)

    # causal mask block: keep (q - k >= 0) else -1e30  [partition=k, free=q]
    ktri = np.arange(128)
    tri_np = np.where(ktri[None, :] - ktri[:, None] >= 0, 0.0, NEG).astype(
        np.float32
    )

    in_maps = []
    for c in range(N_CORES):
        b, g = divmod(c, N_CORES // B)
        cs = slice(GW * g, GW * (g + 1))
        in_maps.append(
            {
                "xT": xTs[b],
                "wq": warr(Wq, cs),
                "wk": warr(Wk, cs),
                "wv": warr(Wv, cs),
                "wo": woarr(cs),
                "tri": tri_np,
            }
        )

    _maybe_install_trace_hook()
    res = bass_utils.run_bass_kernel_spmd(nc, in_maps, core_ids=list(range(N_CORES)))
    LAST_RESULTS = res

    out = np.zeros((B, S, D), dtype=np.float32)
    for c in range(N_CORES):
        out[c // (N_CORES // B)] += res.results[c]["out"].astype(np.float32)
    out += bo[None, None, :]
    return out


# revision 61
# speedup vs baseline: 1.0186x; 1.0063x over previous
"""Trainium2 Bass kernel for multi-head causal attention.

Problem: B=2, S=2048, D=1024, H=16 heads (head_dim=64), fp32.
  q,k,v = x@Wq, x@Wk, x@Wv  (per-head split)
  scores = q@k^T, causal mask, softmax(scores/sqrt(64))
  out = (attn@v concat) @ Wo + bo

Sharding (8 cores): core c -> batch b=c//4, head group g=c%4 (4 heads).
Each core computes its 4 heads' attention plus the partial output
projection (row-parallel Wo); host sums 4 partials per batch and adds bo.

Layout strategy (zero on-device transposes):
 - x^T passed host-transposed (feature-major).
 - Q^T,K^T produced feature-major: (head_dim x tokens), two heads stacked
   per 128-partition tile; scores^T computed per 64-partition row group.
 - Both heads' scores^T tiles (k x q) land in one 2-bank PSUM tile so the
   causal mask add + exp run as single wide instructions. The exp'd
   bf16 tile is directly the PV stationary operand. V is token-major with
   an appended ones-column so the PV matmul also emits the softmax
   denominators.

Scheduling: the attention inner loop is ACT-bound - each k-chunk's exp
(~1.1us) exceeds the PE work for that chunk (~0.65us scores+PV). All
projection/output-projection matmuls are therefore decomposed into
single-matmul work items and PUMPED into the per-chunk PE idle windows
with an ns-budget pacer, so the tensor engine streams continuously while
the scalar engine streams exps. Emission-order rules learned from traces:
 - scores(j) is emitted BEFORE pv(j-2) every iteration: the exp-stream
   critical chain is exp(j-2) ->[sc slot]-> scores(j) -> exp(j); the PV
   pair's consumer is many chunks away.
 - WO chains carry an inflated pump cost so they spread out ~1 chain per
   1.5 js: a dense WO burst makes chain n's psum-slot wait (freed by
   chain n-2's staging copy) back up into the in-order PE queue ahead of
   the attention stream.
 - Cross-pair score prefill between the two drain PVs keeps ACT fed
   across pair boundaries; range-0 pairs emit all scores before any PV
   because the PVs wait on the late wv DMA.
 - gpsimd runs ONLY partition_broadcast: mixing in other ops thrashes
   LOAD_LIB/UNLOAD_LIB microcode swaps with multi-us stalls; gpsimd also
   cannot touch PSUM, so all PSUM->SBUF staging is DVE (+ scalar for the
   o=0 WO half).
Softmax normalization (reciprocal of the ones-column denominators +
gpsimd partition broadcast + DVE multiply) runs off the critical path,
with a PE-row-broadcast fast path for the tail-critical final pair; the
final-range WO chains alternate into the idle scores psum tag for a
4-slot tail pipeline.

PSUM budget (8 banks): scores 2 tiles x 2 banks, PV accumulators 2 x 1,
projection/WO chains 2 x 1.
Matmul operands are bf16 (1 cycle/row PE rate); every accumulation and
the softmax normalization stay fp32 in PSUM.
"""

import sys

sys.path.insert(0, "/opt/trn_rl_repo")

from collections import deque

import ml_dtypes
import numpy as np

import concourse.bass as bass  # noqa: F401
import concourse.tile as tile
from concourse import bacc, bass_utils, mybir

F32 = mybir.dt.float32
MMDT = mybir.dt.bfloat16
NPDT = ml_dtypes.bfloat16
EXPF = mybir.ActivationFunctionType.Exp

B, S, D, H, HD = 2, 2048, 1024, 16, 64
N_CORES = 8
HPC = 4            # heads per core
GW = HPC * HD      # head-group width per core = 256
SCALE = 1.0 / np.sqrt(HD)
NEG = -1.0e30

NT = S // 512      # 4 q/t ranges of 512
NC = D // 128      # 8 contraction chunks for projections
NJ = S // 128      # 16 k-chunks

_CACHE = {}
LAST_RESULTS = None


def _maybe_install_trace_hook():
    """If BASS_TRACE is set, bass_utils needs antenv.axon_hooks (absent in
    this image). Install it from trn_boot when possible; otherwise disable
    tracing so the run still works."""
    import os

    if not os.environ.get("BASS_TRACE"):
        return
    try:
        import antenv.axon_hooks  # noqa: F401
        return
    except ImportError:
        pass
    try:
        import types

        from trn_agent_boot.trn_boot import _ntff_profile_via_ctypes

        hook = _ntff_profile_via_ctypes("/opt/axon/libaxon_pjrt.so")
        mod = types.ModuleType("antenv.axon_hooks")
        mod.get_axon_ntff_profile_hook = lambda: hook
        mod.set_axon_ntff_profile_hook = lambda h: None
        import antenv

        sys.modules["antenv.axon_hooks"] = mod
        antenv.axon_hooks = mod
    except Exception:
        os.environ["BASS_NEVER_TRACE"] = "1"


def _build():
    nc = bacc.Bacc("TRN2", target_bir_lowering=False, debug=False)

    # x is passed host-rearranged range-major: row 128*r+p holds the NC=8
    # contraction chunks for q-range r, each 512 tokens contiguous, so one
    # per-range DMA is 128 descriptors x 8KB.
    xT = nc.dram_tensor("xT", [(S // 512) * 128, (D // 128) * 512], MMDT,
                        kind="ExternalInput").ap()
    wq = nc.dram_tensor("wq", [128, D // 128 * GW], MMDT, kind="ExternalInput").ap()
    wk = nc.dram_tensor("wk", [128, D // 128 * GW], MMDT, kind="ExternalInput").ap()
    wv = nc.dram_tensor("wv", [128, D // 128 * GW], MMDT, kind="ExternalInput").ap()
    wo = nc.dram_tensor("wo", [128, GW // 128 * D], MMDT, kind="ExternalInput").ap()
    trid = nc.dram_tensor("tri", [128, 128], F32, kind="ExternalInput").ap()
    out = nc.dram_tensor("out", [S, D], MMDT, kind="ExternalOutput").ap()

    with tile.TileContext(nc) as tc, nc.allow_low_precision(reason="bf16 matmuls"):
        with (
            tc.tile_pool(name="const", bufs=1) as cpool,
            tc.tile_pool(name="xin", bufs=3) as xpool,
            tc.tile_pool(name="pt", bufs=32) as ppool,
            tc.tile_pool(name="small", bufs=4) as spool,
            tc.tile_pool(name="psum", bufs=1, space="PSUM") as psum,
        ):
            # ---- persistent tiles ----
            wq_sb = cpool.tile([128, NC, GW], MMDT)
            wk_sb = cpool.tile([128, NC, GW], MMDT)
            wv_sb = cpool.tile([128, NC, GW], MMDT)
            wo_sb = cpool.tile([128, 2, D], MMDT)

            QT = cpool.tile([128, 2, S], MMDT)   # [:, pair, t] feature-major
            KT = cpool.tile([128, 2, S], MMDT)
            Vt = cpool.tile([128, NJ, HPC * 128], MMDT)  # token-major + 64 ones cols
            ctxT = cpool.tile([128, 2, S], MMDT)

            # PE warmup: stream constant data through the tensor engine while
            # the first DMAs land, so the DVFS activity monitor ramps the PE
            # clock to max before real matmuls begin. One accumulation group
            # so no inter-instruction semaphores serialize it.
            warm_sb = cpool.tile([128, 512], MMDT, name="warm")
            nc.vector.memset(warm_sb[:], 0.125)
            warm_ps = psum.tile([128, 1024], F32, tag="sc", bufs=2)
            NWARM = 14
            for i in range(NWARM):
                nc.tensor.matmul(
                    warm_ps[:, 0:512], warm_sb[:, 0:128], warm_sb[:],
                    start=(i == 0), stop=(i == NWARM - 1),
                )
            # preload the Exp activation table during the DMA preamble so the
            # first real exp doesn't eat the lazy ACT_TABLE_LOAD.
            tbl = cpool.tile([1, 8], F32, name="tbl")
            nc.scalar.activation(tbl[:], warm_sb[0:1, 0:8], EXPF, scale=SCALE)

            # ones columns 64..127 of each 128-wide head slot: the PV
            # matmul then emits 64 BROADCAST COPIES of the softmax
            # denominator (rows 64:128 of the accumulator) at zero moving
            # cost - stationary width does not affect matmul cycles. This
            # removes the whole bounce->reciprocal->partition-broadcast
            # chain (the reciprocal runs directly on rows 64:128).
            vt_ones = Vt[:, :, :].rearrange("p j (h u) -> p (j h) u", u=128)[:, :, 64:128]
            nc.vector.memset(vt_ones, 1.0)

            # triangular causal mask for the diagonal 128-block of scores^T:
            # keep (q - k >= 0) else -1e30   [partition = k, free = q]
            # (DMA emitted later, after the startup-critical wq/x transfers)
            tri = cpool.tile([128, 128], F32, name="tri")

            # broadcast view of tri over the two stacked heads (0-stride dim)
            tri_ap = tri[:]
            tri2 = bass.AP(
                tensor=tri_ap.tensor,
                offset=tri_ap.offset,
                ap=[list(tri_ap.ap[0]), [0, 2], list(tri_ap.ap[1])],
            )

            xts = {}

            def load_xt(r, split=False):
                xt = xpool.tile([128, NC, 512], MMDT, tag="xt")
                xv = xT[128 * r : 128 * (r + 1), :].rearrange(
                    "p (c t) -> p c t", t=512
                )
                if split:
                    # startup: land the first chunks earlier so the first
                    # projection matmuls can begin sooner.
                    nc.sync.dma_start(xt[:, 0:4, :], xv[:, 0:4, :])
                    nc.sync.dma_start(xt[:, 4:8, :], xv[:, 4:8, :])
                else:
                    nc.sync.dma_start(xt[:], xv)
                xts[r] = xt

            # ---------- filler work-item machinery ----------
            # Each item: (pe_cost_ns, fn). Markers gate force-drains so a
            # consumer can guarantee a producer chain has been emitted.
            fillers = deque()
            markers_done = set()

            def pump(budget):
                while fillers:
                    head = fillers[0]
                    if head[0] is None:
                        fillers.popleft()
                        markers_done.add(head[1])
                        continue
                    if head[0] > budget:
                        break
                    fillers.popleft()
                    head[1]()
                    budget -= head[0]
                return budget

            def drain_until(name):
                if name in markers_done:
                    return
                while fillers:
                    head = fillers.popleft()
                    if head[0] is None:
                        markers_done.add(head[1])
                        if head[1] == name:
                            return
                        continue
                    head[1]()

            def qk_items(r, w_sb, dst, o):
                st = {}
                def item(c):
                    def go():
                        if c == 0:
                            st["pm"] = psum.tile([128, 512], F32, tag="pj", bufs=2, name=f"pm{r}_{o}_{id(st)%97}")
                        nc.tensor.matmul(
                            st["pm"][:, :],
                            w_sb[:, c, 128 * o : 128 * (o + 1)],
                            xts[r][:, c, :],
                            start=(c == 0),
                            stop=(c == NC - 1),
                        )
                        if c == NC - 1:
                            # two half-copies: finer DVE granularity keeps
                            # the diagonal tri-adds from queuing behind a
                            # full-width copy.
                            nc.vector.tensor_copy(
                                dst[:, o, 512 * r : 512 * r + 256],
                                st["pm"][:, 0:256],
                            )
                            nc.vector.tensor_copy(
                                dst[:, o, 512 * r + 256 : 512 * (r + 1)],
                                st["pm"][:, 256:512],
                            )
                    return (216, go)
                return [item(c) for c in range(NC)]

            def v_items(r, tt):
                st = {}
                def item(c):
                    def go():
                        if c == 0:
                            st["pv"] = psum.tile([128, 512], F32, tag="pj", bufs=2, name=f"pvp{r}_{tt}")
                        nc.tensor.matmul(
                            st["pv"][:, 0:GW],
                            xts[r][:, c, 128 * tt : 128 * (tt + 1)],
                            wv_sb[:, c, :],
                            start=(c == 0),
                            stop=(c == NC - 1),
                        )
                        if c == NC - 1:
                            j = 4 * r + tt
                            vtv = Vt[:, j, :].rearrange(
                                "p (h u) -> p h u", u=128)
                            pvv = st["pv"][:, 0:GW].rearrange(
                                "p (h d) -> p h d", d=HD)
                            nc.vector.tensor_copy(
                                vtv[:, 0:2, 0:64], pvv[:, 0:2, :])
                            nc.vector.tensor_copy(
                                vtv[:, 2:4, 0:64], pvv[:, 2:4, :])
                    return (112, go)
                return [item(c) for c in range(NC)]

            def wo_items(r, qq, o, ptag="pj"):
                st = {}
                qt = 4 * r + qq
                def item(d):
                    def go():
                        if d == 0:
                            if ptag == "sc":
                                # tail chains borrow the (idle) scores psum
                                # tag for a deeper WO pipeline.
                                t = psum.tile([128, 1024], F32, tag="sc",
                                              bufs=2, name=f"posc{qt}_{o}")
                                st["po"] = t[:, 0:512]
                            else:
                                st["po"] = psum.tile(
                                    [128, 512], F32, tag="pj", bufs=2,
                                    name=f"po{qt}_{o}")[:, :]
                        nc.tensor.matmul(
                            st["po"],
                            ctxT[:, d, 128 * qt : 128 * (qt + 1)],
                            wo_sb[:, d, 512 * o : 512 * (o + 1)],
                            start=(d == 0), stop=(d == 1),
                        )
                        if d == 1:
                            # stage to SBUF (DMA cannot read PSUM), then DMA
                            # the bf16 partial out; host sums the partials.
                            # Copies alternate scalar/DVE to spread load.
                            ot = spool.tile(
                                [128, 512], MMDT, tag="ot", name=f"ot{qt}_{o}"
                            )
                            if o == 0:
                                nc.scalar.copy(ot[:], st["po"])
                            else:
                                nc.vector.tensor_copy(ot[:, 0:256], st["po"][:, 0:256])
                                nc.vector.tensor_copy(ot[:, 256:512], st["po"][:, 256:512])
                            nc.sync.dma_start(
                                out[128 * qt : 128 * (qt + 1),
                                    512 * o : 512 * (o + 1)],
                                ot[:],
                            )
                    # inflated pump cost: paces WO chains to ~one per 3-4
                    # j-iterations so a chain's psum-slot wait (freed by the
                    # previous chain's copy) never backs up into the in-order
                    # PE queue ahead of the attention stream.
                    return (450, go)
                return [item(d) for d in range(2)]

            def add_a_fillers(r):
                fillers.extend(qk_items(r, wq_sb, QT, 0))
                fillers.append((None, f"Q{r}o0"))
                fillers.extend(qk_items(r, wk_sb, KT, 0))
                fillers.append((None, f"K{r}o0"))
                for tt in range(4):
                    fillers.extend(v_items(r, tt))
                    fillers.append((None, f"V{r}t{tt}"))
                fillers.extend(qk_items(r, wq_sb, QT, 1))
                fillers.extend(qk_items(r, wk_sb, KT, 1))
                fillers.append((None, f"a{r}"))

            def add_c_fillers(r):
                for qq in range(4):
                    for o in range(2):
                        fillers.extend(wo_items(r, qq, o))

            # ---------- attention ----------
            pairs = [(r, p) for r in range(NT) for p in (0, 1)]
            pstate = {}

            def scores(r, p, j):
                # QK^T for both heads of the pair; the two 64-row matmuls run
                # concurrently on disjoint PE row halves.
                v = j - 4 * r
                off = 128 * v if v > 0 else 0   # q cols < off invalid
                if r > 0 and v == 0:
                    drain_until(f"K{r}o0")
                s2 = psum.tile([128, 1024], F32, tag="sc", bufs=2)
                nc.tensor.matmul(
                    s2[:, off:512],
                    KT[0:64, p, 128 * j : 128 * (j + 1)],
                    QT[0:64, p, 512 * r + off : 512 * (r + 1)],
                    start=True, stop=True,
                )
                nc.tensor.matmul(
                    s2[:, 512 + off : 1024],
                    KT[64:128, p, 128 * j : 128 * (j + 1)],
                    QT[64:128, p, 512 * r + off : 512 * (r + 1)],
                    start=True, stop=True,
                )
                pt2 = ppool.tile([128, 1024], MMDT, tag="pt")
                s2v = s2[:, :].rearrange("p (s q) -> p s q", s=2)
                pt2v = pt2[:, :].rearrange("p (s q) -> p s q", s=2)
                if v >= 0:      # diagonal block inside this q-range
                    nc.vector.tensor_add(
                        s2v[:, :, off : off + 128],
                        s2v[:, :, off : off + 128],
                        tri2,
                    )
                nc.scalar.activation(
                    pt2v[:, :, off:512], s2v[:, :, off:512],
                    EXPF, scale=SCALE,
                )
                return pt2, off

            def start_pair(r, p):
                # Two-pass PV: head A accumulates inline in the j-loop; head
                # B's whole PV chain is deferred as pumpable filler (it only
                # needs the persistent pt2 tiles). The LAST TWO pairs stay
                # fully inline so the single accB bank never has two
                # simultaneous users and the kernel tail doesn't grow.
                inline = pairs.index((r, p)) >= len(pairs) - 2
                ca = psum.tile([128, 512], F32, tag="accA", bufs=1,
                               name=f"ca{r}_{p}")
                # the inline cb is allocated lazily at its first write so the
                # accB slot rotation follows true first-write order (the
                # previous pair's deferred pass-B chain allocates first).
                pstate[(r, p)] = {"ca": ca, "cb": None, "pend": deque(),
                                  "bq": [], "inline": inline}

            def prefill_scores(r, p, j):
                pstate[(r, p)]["pend"].append(scores(r, p, j))

            def pv(r, p, j):
                st = pstate[(r, p)]
                pt2, off = st["pend"].popleft()
                v = j - 4 * r
                if r > 0 and 0 <= v < 4:
                    drain_until(f"V{r}t{v}")
                hA, hB = 2 * p, 2 * p + 1
                nj = 4 * r + 4
                nc.tensor.matmul(
                    st["ca"][:, off:512],
                    Vt[:, j, 128 * hA : 128 * hA + 128],
                    pt2[:, off:512],
                    start=(j == 0), stop=(j == nj - 1),
                )
                if st["inline"]:
                    if st["cb"] is None:
                        st["cb"] = psum.tile([128, 512], F32, tag="accB",
                                             bufs=1, name=f"cbi{r}_{p}")
                    nc.tensor.matmul(
                        st["cb"][:, off:512],
                        Vt[:, j, 128 * hB : 128 * hB + 128],
                        pt2[:, 512 + off : 1024],
                        start=(j == 0), stop=(j == nj - 1),
                    )
                else:
                    st["bq"].append((j, pt2, off))

            def normalize_head(r, p, head, acc):
                # stage ctx rows and denominator rows separately (the
                # denominator copy shifts partitions 64:128 -> 0:64, which
                # plain copies support; the custom-DVE reciprocal does NOT
                # support partition-shifted operands, so it runs base-0).
                sth = spool.tile([64, 512], F32, tag="st")
                den = spool.tile([64, 512], F32, tag="sw")
                nc.vector.tensor_copy(sth[:], acc[0:64, :])
                nc.vector.tensor_copy(den[:], acc[64:128, :])
                ra = spool.tile([64, 512], F32, tag="rc")
                nc.vector.reciprocal_approx_fast(ra[:], den[:])
                qs = slice(512 * r, 512 * (r + 1))
                rows = slice(0, 64) if head == 0 else slice(64, 128)
                nc.vector.tensor_mul(ctxT[rows, p, qs], sth[:], ra[:])

            def add_b_fillers(r, p):
                st = pstate[(r, p)]
                hB = 2 * p + 1
                nj = 4 * r + 4
                bst = {}
                def mk(j, pt2, off):
                    def go():
                        if j == 0:
                            bst["cb"] = psum.tile(
                                [128, 512], F32, tag="accB", bufs=1,
                                name=f"cbB{r}_{p}")
                        nc.tensor.matmul(
                            bst["cb"][:, off:512],
                            Vt[:, j, 128 * hB : 128 * hB + 128],
                            pt2[:, 512 + off : 1024],
                            start=(j == 0), stop=(j == nj - 1),
                        )
                    return (max(60, int(0.43 * (512 - off))), go)
                items = [mk(j, pt2, off) for (j, pt2, off) in st["bq"]]
                def normb():
                    normalize_head(r, p, 1, bst["cb"])
                items.append((0, normb))
                items.append((None, f"pB{pairs.index((r, p))}"))
                idxp = pairs.index((r, p))
                if idxp + 1 >= len(pairs) - 2:
                    # the NEXT pair is inline: this chain will be force-
                    # drained at its start. Clear older pass-B (so the accB
                    # allocation order stays first-write) and PREPEND, so the
                    # boundary pump drains most of it under the prefill-exp
                    # cover instead of as an uncovered block.
                    if idxp >= 1:
                        drain_until(f"pB{idxp - 1}")
                    fillers.extendleft(reversed(items))
                else:
                    fillers.extend(items)

            def normalize(r, p, is_last):
                st = pstate[(r, p)]
                ca, cb = st["ca"], st["cb"]
                qs = slice(512 * r, 512 * (r + 1))
                if is_last:
                    # tail-critical pair: denominator rows copied to base-0
                    # (shift via plain copy), reciprocal, multiply straight
                    # from PSUM - the banks are not needed again.
                    dnA = spool.tile([64, 512], F32, tag="sw")
                    dnB = spool.tile([64, 512], F32, tag="sw")
                    nc.vector.tensor_copy(dnA[:], ca[64:128, :])
                    nc.vector.tensor_copy(dnB[:], cb[64:128, :])
                    ra = spool.tile([64, 512], F32, tag="rc")
                    rb = spool.tile([64, 512], F32, tag="rc")
                    nc.vector.reciprocal_approx_fast(ra[:], dnA[:])
                    nc.vector.reciprocal_approx_fast(rb[:], dnB[:])
                    nc.vector.tensor_mul(ctxT[0:64, p, qs], ca[0:64, :], ra[:])
                    nc.vector.tensor_mul(ctxT[64:128, p, qs], cb[0:64, :], rb[:])
                    return
                normalize_head(r, p, 0, ca)
                normalize_head(r, p, 1, cb)

            # ---------- startup ----------
            # DMA order is startup-critical: wq + first x chunks first so the
            # first Q-projection matmuls start as early as possible.
            wqv = wq.rearrange("p (c o) -> p c o", o=GW)
            nc.sync.dma_start(wq_sb[:], wqv)
            load_xt(0, split=True)
            nc.sync.dma_start(wk_sb[:], wk.rearrange("p (c o) -> p c o", o=GW))
            nc.sync.dma_start(tri[:], trid)
            nc.sync.dma_start(wv_sb[:], wv.rearrange("p (c o) -> p c o", o=GW))
            load_xt(1)
            nc.sync.dma_start(wo_sb[:], wo.rearrange("p (c o) -> p c o", o=D))

            # Only the pair-(0,0)-critical projections run before attention
            # starts; everything else becomes pumpable filler.
            for it in qk_items(0, wq_sb, QT, 0):
                it[1]()
            for it in qk_items(0, wk_sb, KT, 0):
                it[1]()

            # ---------- main pair loop ----------
            BUDGET_CAP = 2600
            PAIR_BONUS = 1100

            start_pair(0, 0)
            prefill_scores(0, 0, 0)
            prefill_scores(0, 0, 1)

            # pair-(0,1) projections BEFORE the V block: they only need
            # wq/wk/xt0 (landed), while V waits on the later wv DMA - V at
            # the queue head would stall everything behind it.
            for it in qk_items(0, wq_sb, QT, 1):
                it[1]()
            for it in qk_items(0, wk_sb, KT, 1):
                it[1]()
            markers_done.add("Q0o0")
            markers_done.add("K0o0")
            markers_done.add("a0")

            def finish_pair(r, p, idx):
                st = pstate[(r, p)]
                if st["inline"]:
                    normalize(r, p, is_last=(idx == len(pairs) - 1))
                else:
                    normalize_head(r, p, 0, st["ca"])
                    add_b_fillers(r, p)

            budget = 0
            for idx, (r, p) in enumerate(pairs):
                nj = 4 * r + 4
                if idx >= 2:
                    # bound pt2-pool pressure: pass B of the pair before last
                    # must be fully emitted before this pair's scores flood
                    # the pool with new exp outputs.
                    drain_until(f"pB{idx - 2}")
                if idx >= 1 and pstate[(r, p)]["inline"] \
                        and not pstate[pairs[idx - 1]]["inline"]:
                    # inline pair: the previous pair's pass-B chain must be
                    # fully emitted first, or its accB writes queue behind
                    # this pair's inline cb use -> PE-queue deadlock.
                    drain_until(f"pB{idx - 1}")
                if p == 0:
                    if r + 2 < NT:
                        load_xt(r + 2)
                    if r + 1 < NT:
                        add_a_fillers(r + 1)
                    # WO deferred by two ranges: range 2's pump capacity is
                    # fully claimed by the a3 projection chains; the two-pass
                    # j-loops of range 3 have spare capacity for the WO load.
                    if r >= 2:
                        add_c_fillers(r - 2)
                    if r == NT - 1:
                        add_c_fillers(r - 1)
                budget = min(budget + PAIR_BONUS, BUDGET_CAP)
                nxt = pairs[idx + 1] if idx + 1 < len(pairs) else None
                if r == 0:
                    # range-0 pairs (nj=4): the PVs wait on the late wv DMA;
                    # emit ALL scores and the next pair's prefill before any
                    # PV so the exp stream never queues behind V-gated work.
                    for j in (2, 3):
                        pstate[(r, p)]["pend"].append(scores(r, p, j))
                    if nxt is not None:
                        if nxt[0] != r:
                            drain_until(f"Q{nxt[0]}o0")
                        start_pair(*nxt)
                        prefill_scores(*nxt, 0)
                        prefill_scores(*nxt, 1)
                    if p == 0:
                        # V tiles emitted only now: wv lands late; anything
                        # queued behind a V matmul would stall the exp chain.
                        for tt in range(4):
                            for it in v_items(0, tt):
                                it[1]()
                            markers_done.add(f"V0t{tt}")
                    for j in range(nj):
                        pv(r, p, j)
                    finish_pair(r, p, idx)
                    budget = pump(min(budget + 2800, 4800))
                    continue
                for j in range(2, nj):
                    # scores(j) FIRST: it is on the exp-stream critical chain
                    # (slot freed by exp(j-2) -> scores(j) -> exp(j)); the PV
                    # pair's consumer is many js away, so it follows.
                    pstate[(r, p)]["pend"].append(scores(r, p, j))
                    pv(r, p, j - 2)
                    w = 512 - (128 * (j - 4 * r) if j - 4 * r > 0 else 0)
                    inc = (int(0.35 * w + 210) if pstate[(r, p)]["inline"]
                           else int(0.68 * w + 235))
                    budget = pump(min(budget + inc, 3600))
                # tail: drain last two PVs, prefilling the next pair's first
                # two score chunks in between so ACT never starves.
                if nxt is not None:
                    if nxt[0] != r:
                        drain_until(f"Q{nxt[0]}o0")
                    else:
                        # pair 1's prefill reads QT/KT o=1 of this range:
                        # force those chains out before emitting the reads.
                        drain_until(f"a{r}")
                    start_pair(*nxt)
                pv(r, p, nj - 2)
                if nxt is not None:
                    prefill_scores(*nxt, 0)
                pv(r, p, nj - 1)
                if nxt is not None:
                    prefill_scores(*nxt, 1)
                finish_pair(r, p, idx)
                # boundary window: the next pair's first two exps cover
                # ~2.2us of PE time - generous allowance drains a WO chain.
                budget = pump(min(budget + 2800, 4800))

            # Final drain: leftover fillers first, then the last range's WO
            # chains alternating pj/sc psum tags so FOUR slots keep the tail
            # pipeline deep. The first two chains' d=0 matmuls (pair-0 ctxT,
            # already normalized) are emitted up front so the PE stays busy
            # (and clocked up) through the final normalization latency.
            drain_until("__all__")
            chains = [
                wo_items(NT - 1, qq, o,
                         ptag=("pj" if (2 * qq + o) % 2 == 0 else "sc"))
                for qq in range(4) for o in range(2)
            ]
            for ch in chains[:2]:
                ch[0][1]()
            for ch in chains[:2]:
                ch[1][1]()
            for ch in chains[2:]:
                ch[0][1]()
                ch[1][1]()

    nc.compile()
    return nc


def _get_nc():
    if "nc" not in _CACHE:
        _CACHE["nc"] = _build()
    return _CACHE["nc"]


def kernel(x, Wq, Wk, Wv, Wo, bo):
    global LAST_RESULTS
    x = np.asarray(x, dtype=np.float32)
    Wq = np.asarray(Wq, dtype=np.float32)
    Wk = np.asarray(Wk, dtype=np.float32)
    Wv = np.asarray(Wv, dtype=np.float32)
    Wo = np.asarray(Wo, dtype=np.float32)
    bo = np.asarray(bo, dtype=np.float32)

    nc = _get_nc()
    # range-major layout: [NT*128, NC*512]; row 128*r+p holds chunks c=0..7
    # (512 tokens each, contiguous) of q-range r for feature-row p.

    def xarr(b):
        a = x[b].T.reshape(NC, 128, NT, 512).transpose(2, 1, 0, 3)
        return np.ascontiguousarray(a.reshape(NT * 128, NC * 512)).astype(NPDT)

    xTs = [xarr(b) for b in range(B)]

    def warr(w, cs):
        # [D, GW] slice -> [128, NC*GW]: partition p holds chunk-major rows
        s = w[:, cs].reshape(D // 128, 128, GW).transpose(1, 0, 2)
        return np.ascontiguousarray(s.reshape(128, -1)).astype(NPDT)

    def woarr(cs):
        # [GW, D] slice -> [128, 2*D]
        s = Wo[cs, :].reshape(GW // 128, 128, D).transpose(1, 0, 2)
        return np.ascontiguousarray(s.reshape(128, -1)).astype(NPDT)

    # causal mask block: keep (q - k >= 0) else -1e30  [partition=k, free=q]
    ktri = np.arange(128)
    tri_np = np.where(ktri[None, :] - ktri[:, None] >= 0, 0.0, NEG).astype(
        np.float32
    )

    in_maps = []
    for c in range(N_CORES):
        b, g = divmod(c, N_CORES // B)
        cs = slice(GW * g, GW * (g + 1))
        in_maps.append(
            {
                "xT": xTs[b],
                "wq": warr(Wq, cs),
                "wk": warr(Wk, cs),
                "wv": warr(Wv, cs),
                "wo": woarr(cs),
                "tri": tri_np,
            }
        )

    _maybe_install_trace_hook()
    res = bass_utils.run_bass_kernel_spmd(nc, in_maps, core_ids=list(range(N_CORES)))
    LAST_RESULTS = res

    out = np.zeros((B, S, D), dtype=np.float32)
    for c in range(N_CORES):
        out[c // (N_CORES // B)] += res.results[c]["out"].astype(np.float32)
    out += bo[None, None, :]
    return out


# revision 62
# speedup vs baseline: 1.0270x; 1.0083x over previous
"""Trainium2 Bass kernel for multi-head causal attention.

Problem: B=2, S=2048, D=1024, H=16 heads (head_dim=64), fp32.
  q,k,v = x@Wq, x@Wk, x@Wv  (per-head split)
  scores = q@k^T, causal mask, softmax(scores/sqrt(64))
  out = (attn@v concat) @ Wo + bo

Sharding (8 cores): core c -> batch b=c//4, head group g=c%4 (4 heads).
Each core computes its 4 heads' attention plus the partial output
projection (row-parallel Wo); host sums 4 partials per batch and adds bo.

Layout strategy (zero on-device transposes):
 - x^T passed host-transposed (feature-major).
 - Q^T,K^T produced feature-major: (head_dim x tokens), two heads stacked
   per 128-partition tile; scores^T computed per 64-partition row group.
 - Both heads' scores^T tiles (k x q) land in one 2-bank PSUM tile so the
   causal mask add + exp run as single wide instructions. The exp'd
   bf16 tile is directly the PV stationary operand. V is token-major with
   an appended ones-column so the PV matmul also emits the softmax
   denominators.

Scheduling: the attention inner loop is ACT-bound - each k-chunk's exp
(~1.1us) exceeds the PE work for that chunk (~0.65us scores+PV). All
projection/output-projection matmuls are therefore decomposed into
single-matmul work items and PUMPED into the per-chunk PE idle windows
with an ns-budget pacer, so the tensor engine streams continuously while
the scalar engine streams exps. Emission-order rules learned from traces:
 - scores(j) is emitted BEFORE pv(j-2) every iteration: the exp-stream
   critical chain is exp(j-2) ->[sc slot]-> scores(j) -> exp(j); the PV
   pair's consumer is many chunks away.
 - WO chains carry an inflated pump cost so they spread out ~1 chain per
   1.5 js: a dense WO burst makes chain n's psum-slot wait (freed by
   chain n-2's staging copy) back up into the in-order PE queue ahead of
   the attention stream.
 - Cross-pair score prefill between the two drain PVs keeps ACT fed
   across pair boundaries; range-0 pairs emit all scores before any PV
   because the PVs wait on the late wv DMA.
 - gpsimd runs ONLY partition_broadcast: mixing in other ops thrashes
   LOAD_LIB/UNLOAD_LIB microcode swaps with multi-us stalls; gpsimd also
   cannot touch PSUM, so all PSUM->SBUF staging is DVE (+ scalar for the
   o=0 WO half).
Softmax normalization (reciprocal of the ones-column denominators +
gpsimd partition broadcast + DVE multiply) runs off the critical path,
with a PE-row-broadcast fast path for the tail-critical final pair; the
final-range WO chains alternate into the idle scores psum tag for a
4-slot tail pipeline.

PSUM budget (8 banks): scores 2 tiles x 2 banks, PV accumulators 2 x 1,
projection/WO chains 2 x 1.
Matmul operands are bf16 (1 cycle/row PE rate); every accumulation and
the softmax normalization stay fp32 in PSUM.
"""

import sys

sys.path.insert(0, "/opt/trn_rl_repo")

from collections import deque

import ml_dtypes
import numpy as np

import concourse.bass as bass  # noqa: F401
import concourse.tile as tile
from concourse import bacc, bass_utils, mybir

F32 = mybir.dt.float32
MMDT = mybir.dt.bfloat16
NPDT = ml_dtypes.bfloat16
EXPF = mybir.ActivationFunctionType.Exp

B, S, D, H, HD = 2, 2048, 1024, 16, 64
N_CORES = 8
HPC = 4            # heads per core
GW = HPC * HD      # head-group width per core = 256
SCALE = 1.0 / np.sqrt(HD)
NEG = -1.0e30

NT = S // 512      # 4 q/t ranges of 512
NC = D // 128      # 8 contraction chunks for projections
NJ = S // 128      # 16 k-chunks

_CACHE = {}
LAST_RESULTS = None


def _maybe_install_trace_hook():
    """If BASS_TRACE is set, bass_utils needs antenv.axon_hooks (absent in
    this image). Install it from trn_boot when possible; otherwise disable
    tracing so the run still works."""
    import os

    if not os.environ.get("BASS_TRACE"):
        return
    try:
        import antenv.axon_hooks  # noqa: F401
        return
    except ImportError:
        pass
    try:
        import types

        from trn_agent_boot.trn_boot import _ntff_profile_via_ctypes

        hook = _ntff_profile_via_ctypes("/opt/axon/libaxon_pjrt.so")
        mod = types.ModuleType("antenv.axon_hooks")
        mod.get_axon_ntff_profile_hook = lambda: hook
        mod.set_axon_ntff_profile_hook = lambda h: None
        import antenv

        sys.modules["antenv.axon_hooks"] = mod
        antenv.axon_hooks = mod
    except Exception:
        os.environ["BASS_NEVER_TRACE"] = "1"


def _build():
    nc = bacc.Bacc("TRN2", target_bir_lowering=False, debug=False)

    # x is passed host-rearranged range-major: row 128*r+p holds the NC=8
    # contraction chunks for q-range r, each 512 tokens contiguous, so one
    # per-range DMA is 128 descriptors x 8KB.
    xT = nc.dram_tensor("xT", [(S // 512) * 128, (D // 128) * 512], MMDT,
                        kind="ExternalInput").ap()
    wq = nc.dram_tensor("wq", [128, D // 128 * GW], MMDT, kind="ExternalInput").ap()
    wk = nc.dram_tensor("wk", [128, D // 128 * GW], MMDT, kind="ExternalInput").ap()
    wv = nc.dram_tensor("wv", [128, D // 128 * GW], MMDT, kind="ExternalInput").ap()
    wo = nc.dram_tensor("wo", [128, GW // 128 * D], MMDT, kind="ExternalInput").ap()
    trid = nc.dram_tensor("tri", [128, 128], F32, kind="ExternalInput").ap()
    out = nc.dram_tensor("out", [S, D], MMDT, kind="ExternalOutput").ap()

    with tile.TileContext(nc) as tc, nc.allow_low_precision(reason="bf16 matmuls"):
        with (
            tc.tile_pool(name="const", bufs=1) as cpool,
            tc.tile_pool(name="xin", bufs=3) as xpool,
            tc.tile_pool(name="pt", bufs=32) as ppool,
            tc.tile_pool(name="small", bufs=4) as spool,
            tc.tile_pool(name="psum", bufs=1, space="PSUM") as psum,
        ):
            # ---- persistent tiles ----
            wq_sb = cpool.tile([128, NC, GW], MMDT)
            wk_sb = cpool.tile([128, NC, GW], MMDT)
            wv_sb = cpool.tile([128, NC, GW], MMDT)
            wo_sb = cpool.tile([128, 2, D], MMDT)

            QT = cpool.tile([128, 2, S], MMDT)   # [:, pair, t] feature-major
            KT = cpool.tile([128, 2, S], MMDT)
            Vt = cpool.tile([128, NJ, HPC * 128], MMDT)  # token-major + 64 ones cols
            ctxT = cpool.tile([128, 2, S], MMDT)

            # PE warmup: stream constant data through the tensor engine while
            # the first DMAs land, so the DVFS activity monitor ramps the PE
            # clock to max before real matmuls begin. One accumulation group
            # so no inter-instruction semaphores serialize it.
            warm_sb = cpool.tile([128, 512], MMDT, name="warm")
            nc.vector.memset(warm_sb[:], 0.125)
            warm_ps = psum.tile([128, 1024], F32, tag="sc", bufs=2)
            NWARM = 14
            for i in range(NWARM):
                nc.tensor.matmul(
                    warm_ps[:, 0:512], warm_sb[:, 0:128], warm_sb[:],
                    start=(i == 0), stop=(i == NWARM - 1),
                )
            # preload the Exp activation table during the DMA preamble so the
            # first real exp doesn't eat the lazy ACT_TABLE_LOAD.
            tbl = cpool.tile([1, 8], F32, name="tbl")
            nc.scalar.activation(tbl[:], warm_sb[0:1, 0:8], EXPF, scale=SCALE)

            # ones columns 64..127 of each 128-wide head slot: the PV
            # matmul then emits 64 BROADCAST COPIES of the softmax
            # denominator (rows 64:128 of the accumulator) at zero moving
            # cost - stationary width does not affect matmul cycles. This
            # removes the whole bounce->reciprocal->partition-broadcast
            # chain (the reciprocal runs directly on rows 64:128).
            vt_ones = Vt[:, :, :].rearrange("p j (h u) -> p (j h) u", u=128)[:, :, 64:128]
            nc.vector.memset(vt_ones, 1.0)

            # triangular causal mask for the diagonal 128-block of scores^T:
            # keep (q - k >= 0) else -1e30   [partition = k, free = q]
            # (DMA emitted later, after the startup-critical wq/x transfers)
            tri = cpool.tile([128, 128], F32, name="tri")

            # broadcast view of tri over the two stacked heads (0-stride dim)
            tri_ap = tri[:]
            tri2 = bass.AP(
                tensor=tri_ap.tensor,
                offset=tri_ap.offset,
                ap=[list(tri_ap.ap[0]), [0, 2], list(tri_ap.ap[1])],
            )

            xts = {}

            def load_xt(r, split=False):
                xt = xpool.tile([128, NC, 512], MMDT, tag="xt")
                xv = xT[128 * r : 128 * (r + 1), :].rearrange(
                    "p (c t) -> p c t", t=512
                )
                if split:
                    # startup: land the first chunks earlier so the first
                    # projection matmuls can begin sooner.
                    nc.sync.dma_start(xt[:, 0:4, :], xv[:, 0:4, :])
                    nc.sync.dma_start(xt[:, 4:8, :], xv[:, 4:8, :])
                else:
                    nc.sync.dma_start(xt[:], xv)
                xts[r] = xt

            # ---------- filler work-item machinery ----------
            # Each item: (pe_cost_ns, fn). Markers gate force-drains so a
            # consumer can guarantee a producer chain has been emitted.
            fillers = deque()
            markers_done = set()

            def pump(budget):
                while fillers:
                    head = fillers[0]
                    if head[0] is None:
                        fillers.popleft()
                        markers_done.add(head[1])
                        continue
                    if head[0] > budget:
                        break
                    fillers.popleft()
                    head[1]()
                    budget -= head[0]
                return budget

            def drain_until(name):
                if name in markers_done:
                    return
                while fillers:
                    head = fillers.popleft()
                    if head[0] is None:
                        markers_done.add(head[1])
                        if head[1] == name:
                            return
                        continue
                    head[1]()

            def qk_items(r, w_sb, dst, o):
                st = {}
                def item(c):
                    def go():
                        if c == 0:
                            st["pm"] = psum.tile([128, 512], F32, tag="pj", bufs=2, name=f"pm{r}_{o}_{id(st)%97}")
                        nc.tensor.matmul(
                            st["pm"][:, :],
                            w_sb[:, c, 128 * o : 128 * (o + 1)],
                            xts[r][:, c, :],
                            start=(c == 0),
                            stop=(c == NC - 1),
                        )
                        if c == NC - 1:
                            # two half-copies: finer DVE granularity keeps
                            # the diagonal tri-adds from queuing behind a
                            # full-width copy.
                            nc.vector.tensor_copy(
                                dst[:, o, 512 * r : 512 * r + 256],
                                st["pm"][:, 0:256],
                            )
                            nc.vector.tensor_copy(
                                dst[:, o, 512 * r + 256 : 512 * (r + 1)],
                                st["pm"][:, 256:512],
                            )
                    return (216, go)
                return [item(c) for c in range(NC)]

            def v_items(r, tt):
                st = {}
                def item(c):
                    def go():
                        if c == 0:
                            st["pv"] = psum.tile([128, 512], F32, tag="pj", bufs=2, name=f"pvp{r}_{tt}")
                        nc.tensor.matmul(
                            st["pv"][:, 0:GW],
                            xts[r][:, c, 128 * tt : 128 * (tt + 1)],
                            wv_sb[:, c, :],
                            start=(c == 0),
                            stop=(c == NC - 1),
                        )
                        if c == NC - 1:
                            j = 4 * r + tt
                            vtv = Vt[:, j, :].rearrange(
                                "p (h u) -> p h u", u=128)
                            pvv = st["pv"][:, 0:GW].rearrange(
                                "p (h d) -> p h d", d=HD)
                            nc.vector.tensor_copy(
                                vtv[:, 0:2, 0:64], pvv[:, 0:2, :])
                            nc.vector.tensor_copy(
                                vtv[:, 2:4, 0:64], pvv[:, 2:4, :])
                    return (112, go)
                return [item(c) for c in range(NC)]

            def wo_items(r, qq, o, ptag="pj"):
                st = {}
                qt = 4 * r + qq
                def item(d):
                    def go():
                        if d == 0:
                            if ptag == "sc":
                                # tail chains borrow the (idle) scores psum
                                # tag for a deeper WO pipeline.
                                t = psum.tile([128, 1024], F32, tag="sc",
                                              bufs=2, name=f"posc{qt}_{o}")
                                st["po"] = t[:, 0:512]
                            else:
                                st["po"] = psum.tile(
                                    [128, 512], F32, tag="pj", bufs=2,
                                    name=f"po{qt}_{o}")[:, :]
                        nc.tensor.matmul(
                            st["po"],
                            ctxT[:, d, 128 * qt : 128 * (qt + 1)],
                            wo_sb[:, d, 512 * o : 512 * (o + 1)],
                            start=(d == 0), stop=(d == 1),
                        )
                        if d == 1:
                            # stage to SBUF (DMA cannot read PSUM), then DMA
                            # the bf16 partial out; host sums the partials.
                            # Copies alternate scalar/DVE to spread load.
                            ot = spool.tile(
                                [128, 512], MMDT, tag="ot", name=f"ot{qt}_{o}"
                            )
                            if o == 0:
                                nc.scalar.copy(ot[:], st["po"])
                            else:
                                nc.vector.tensor_copy(ot[:, 0:256], st["po"][:, 0:256])
                                nc.vector.tensor_copy(ot[:, 256:512], st["po"][:, 256:512])
                            nc.sync.dma_start(
                                out[128 * qt : 128 * (qt + 1),
                                    512 * o : 512 * (o + 1)],
                                ot[:],
                            )
                    # inflated pump cost: paces WO chains to ~one per 3-4
                    # j-iterations so a chain's psum-slot wait (freed by the
                    # previous chain's copy) never backs up into the in-order
                    # PE queue ahead of the attention stream.
                    return (450, go)
                return [item(d) for d in range(2)]

            def add_a_fillers(r):
                fillers.extend(qk_items(r, wq_sb, QT, 0))
                fillers.append((None, f"Q{r}o0"))
                fillers.extend(qk_items(r, wk_sb, KT, 0))
                fillers.append((None, f"K{r}o0"))
                for tt in range(4):
                    fillers.extend(v_items(r, tt))
                    fillers.append((None, f"V{r}t{tt}"))
                fillers.extend(qk_items(r, wq_sb, QT, 1))
                fillers.extend(qk_items(r, wk_sb, KT, 1))
                fillers.append((None, f"a{r}"))

            def add_c_fillers(r):
                for qq in range(4):
                    for o in range(2):
                        fillers.extend(wo_items(r, qq, o))

            # ---------- attention ----------
            pairs = [(r, p) for r in range(NT) for p in (0, 1)]
            pstate = {}

            def scores(r, p, j):
                # QK^T for both heads of the pair; the two 64-row matmuls run
                # concurrently on disjoint PE row halves.
                v = j - 4 * r
                off = 128 * v if v > 0 else 0   # q cols < off invalid
                if r > 0 and v == 0:
                    drain_until(f"K{r}o0")
                s2 = psum.tile([128, 1024], F32, tag="sc", bufs=2)
                nc.tensor.matmul(
                    s2[:, off:512],
                    KT[0:64, p, 128 * j : 128 * (j + 1)],
                    QT[0:64, p, 512 * r + off : 512 * (r + 1)],
                    start=True, stop=True,
                )
                nc.tensor.matmul(
                    s2[:, 512 + off : 1024],
                    KT[64:128, p, 128 * j : 128 * (j + 1)],
                    QT[64:128, p, 512 * r + off : 512 * (r + 1)],
                    start=True, stop=True,
                )
                pt2 = ppool.tile([128, 1024], MMDT, tag="pt")
                s2v = s2[:, :].rearrange("p (s q) -> p s q", s=2)
                pt2v = pt2[:, :].rearrange("p (s q) -> p s q", s=2)
                if v >= 0:      # diagonal block inside this q-range
                    nc.vector.tensor_add(
                        s2v[:, :, off : off + 128],
                        s2v[:, :, off : off + 128],
                        tri2,
                    )
                nc.scalar.activation(
                    pt2v[:, :, off:512], s2v[:, :, off:512],
                    EXPF, scale=SCALE,
                )
                return pt2, off

            def start_pair(r, p):
                # Two-pass PV: head A accumulates inline in the j-loop; head
                # B's whole PV chain is deferred as pumpable filler (it only
                # needs the persistent pt2 tiles). The LAST TWO pairs stay
                # fully inline so the single accB bank never has two
                # simultaneous users and the kernel tail doesn't grow.
                inline = pairs.index((r, p)) >= len(pairs) - 2
                ca = psum.tile([128, 512], F32, tag="accA", bufs=1,
                               name=f"ca{r}_{p}")
                # the inline cb is allocated lazily at its first write so the
                # accB slot rotation follows true first-write order (the
                # previous pair's deferred pass-B chain allocates first).
                pstate[(r, p)] = {"ca": ca, "cb": None, "pend": deque(),
                                  "bq": [], "inline": inline}

            def prefill_scores(r, p, j):
                pstate[(r, p)]["pend"].append(scores(r, p, j))

            def pv(r, p, j):
                st = pstate[(r, p)]
                pt2, off = st["pend"].popleft()
                v = j - 4 * r
                if r > 0 and 0 <= v < 4:
                    drain_until(f"V{r}t{v}")
                hA, hB = 2 * p, 2 * p + 1
                nj = 4 * r + 4
                nc.tensor.matmul(
                    st["ca"][:, off:512],
                    Vt[:, j, 128 * hA : 128 * hA + 128],
                    pt2[:, off:512],
                    start=(j == 0), stop=(j == nj - 1),
                )
                if st["inline"]:
                    if st["cb"] is None:
                        st["cb"] = psum.tile([128, 512], F32, tag="accB",
                                             bufs=1, name=f"cbi{r}_{p}")
                    nc.tensor.matmul(
                        st["cb"][:, off:512],
                        Vt[:, j, 128 * hB : 128 * hB + 128],
                        pt2[:, 512 + off : 1024],
                        start=(j == 0), stop=(j == nj - 1),
                    )
                else:
                    st["bq"].append((j, pt2, off))

            def normalize_head(r, p, head, acc):
                # stage ctx rows and denominator rows separately (the
                # denominator copy shifts partitions 64:128 -> 0:64, which
                # plain copies support; the custom-DVE reciprocal does NOT
                # support partition-shifted operands, so it runs base-0).
                sth = spool.tile([64, 512], F32, tag="st")
                den = spool.tile([64, 512], F32, tag="sw")
                nc.vector.tensor_copy(sth[:], acc[0:64, :])
                nc.vector.tensor_copy(den[:], acc[64:128, :])
                ra = spool.tile([64, 512], F32, tag="rc")
                nc.vector.reciprocal_approx_fast(ra[:], den[:])
                qs = slice(512 * r, 512 * (r + 1))
                rows = slice(0, 64) if head == 0 else slice(64, 128)
                nc.vector.tensor_mul(ctxT[rows, p, qs], sth[:], ra[:])

            def add_b_fillers(r, p):
                st = pstate[(r, p)]
                hB = 2 * p + 1
                nj = 4 * r + 4
                bst = {}
                def mk(j, pt2, off):
                    def go():
                        if j == 0:
                            bst["cb"] = psum.tile(
                                [128, 512], F32, tag="accB", bufs=1,
                                name=f"cbB{r}_{p}")
                        nc.tensor.matmul(
                            bst["cb"][:, off:512],
                            Vt[:, j, 128 * hB : 128 * hB + 128],
                            pt2[:, 512 + off : 1024],
                            start=(j == 0), stop=(j == nj - 1),
                        )
                    return (max(60, int(0.43 * (512 - off))), go)
                items = [mk(j, pt2, off) for (j, pt2, off) in st["bq"]]
                def normb():
                    normalize_head(r, p, 1, bst["cb"])
                items.append((0, normb))
                items.append((None, f"pB{pairs.index((r, p))}"))
                idxp = pairs.index((r, p))
                if idxp + 1 >= len(pairs) - 2:
                    # the NEXT pair is inline: this chain will be force-
                    # drained at its start. Clear older pass-B (so the accB
                    # allocation order stays first-write) and PREPEND, so the
                    # boundary pump drains most of it under the prefill-exp
                    # cover instead of as an uncovered block.
                    if idxp >= 1:
                        drain_until(f"pB{idxp - 1}")
                    fillers.extendleft(reversed(items))
                else:
                    fillers.extend(items)

            def normalize(r, p, is_last):
                st = pstate[(r, p)]
                ca, cb = st["ca"], st["cb"]
                qs = slice(512 * r, 512 * (r + 1))
                if is_last:
                    # tail-critical pair: denominator rows copied to base-0
                    # (shift via plain copy), reciprocal, multiply straight
                    # from PSUM - the banks are not needed again.
                    dnA = spool.tile([64, 512], F32, tag="sw")
                    dnB = spool.tile([64, 512], F32, tag="sw")
                    # parallel copies: scalar engine is exp-free by now
                    nc.scalar.copy(dnA[:], ca[64:128, :])
                    nc.vector.tensor_copy(dnB[:], cb[64:128, :])
                    ra = spool.tile([64, 512], F32, tag="rc")
                    rb = spool.tile([64, 512], F32, tag="rc")
                    nc.vector.reciprocal_approx_fast(ra[:], dnA[:])
                    nc.vector.reciprocal_approx_fast(rb[:], dnB[:])
                    nc.vector.tensor_mul(ctxT[0:64, p, qs], ca[0:64, :], ra[:])
                    nc.vector.tensor_mul(ctxT[64:128, p, qs], cb[0:64, :], rb[:])
                    return
                normalize_head(r, p, 0, ca)
                normalize_head(r, p, 1, cb)

            # ---------- startup ----------
            # DMA order is startup-critical: wq + first x chunks first so the
            # first Q-projection matmuls start as early as possible.
            wqv = wq.rearrange("p (c o) -> p c o", o=GW)
            nc.sync.dma_start(wq_sb[:], wqv)
            load_xt(0, split=True)
            nc.sync.dma_start(wk_sb[:], wk.rearrange("p (c o) -> p c o", o=GW))
            nc.sync.dma_start(tri[:], trid)
            nc.sync.dma_start(wv_sb[:], wv.rearrange("p (c o) -> p c o", o=GW))
            load_xt(1)
            nc.sync.dma_start(wo_sb[:], wo.rearrange("p (c o) -> p c o", o=D))

            # Only the pair-(0,0)-critical projections run before attention
            # starts; everything else becomes pumpable filler.
            for it in qk_items(0, wq_sb, QT, 0):
                it[1]()
            for it in qk_items(0, wk_sb, KT, 0):
                it[1]()

            # ---------- main pair loop ----------
            BUDGET_CAP = 2600
            PAIR_BONUS = 1100

            start_pair(0, 0)
            prefill_scores(0, 0, 0)
            prefill_scores(0, 0, 1)

            # pair-(0,1) projections BEFORE the V block: they only need
            # wq/wk/xt0 (landed), while V waits on the later wv DMA - V at
            # the queue head would stall everything behind it.
            for it in qk_items(0, wq_sb, QT, 1):
                it[1]()
            for it in qk_items(0, wk_sb, KT, 1):
                it[1]()
            markers_done.add("Q0o0")
            markers_done.add("K0o0")
            markers_done.add("a0")

            def finish_pair(r, p, idx):
                st = pstate[(r, p)]
                if st["inline"]:
                    normalize(r, p, is_last=(idx == len(pairs) - 1))
                else:
                    normalize_head(r, p, 0, st["ca"])
                    add_b_fillers(r, p)

            budget = 0
            for idx, (r, p) in enumerate(pairs):
                nj = 4 * r + 4
                if idx >= 2:
                    # bound pt2-pool pressure: pass B of the pair before last
                    # must be fully emitted before this pair's scores flood
                    # the pool with new exp outputs.
                    drain_until(f"pB{idx - 2}")
                if idx >= 1 and pstate[(r, p)]["inline"] \
                        and not pstate[pairs[idx - 1]]["inline"]:
                    # inline pair: the previous pair's pass-B chain must be
                    # fully emitted first, or its accB writes queue behind
                    # this pair's inline cb use -> PE-queue deadlock.
                    drain_until(f"pB{idx - 1}")
                if p == 0:
                    if r + 2 < NT:
                        load_xt(r + 2)
                    if r + 1 < NT:
                        add_a_fillers(r + 1)
                    # WO deferred by two ranges: range 2's pump capacity is
                    # fully claimed by the a3 projection chains; the two-pass
                    # j-loops of range 3 have spare capacity for the WO load.
                    if r >= 2:
                        add_c_fillers(r - 2)
                    if r == NT - 1:
                        add_c_fillers(r - 1)
                budget = min(budget + PAIR_BONUS, BUDGET_CAP)
                nxt = pairs[idx + 1] if idx + 1 < len(pairs) else None
                if r == 0:
                    # range-0 pairs (nj=4): the PVs wait on the late wv DMA;
                    # emit ALL scores and the next pair's prefill before any
                    # PV so the exp stream never queues behind V-gated work.
                    for j in (2, 3):
                        pstate[(r, p)]["pend"].append(scores(r, p, j))
                    if nxt is not None:
                        if nxt[0] != r:
                            drain_until(f"Q{nxt[0]}o0")
                        start_pair(*nxt)
                        prefill_scores(*nxt, 0)
                        prefill_scores(*nxt, 1)
                    if p == 0:
                        # V tiles emitted only now: wv lands late; anything
                        # queued behind a V matmul would stall the exp chain.
                        for tt in range(4):
                            for it in v_items(0, tt):
                                it[1]()
                            markers_done.add(f"V0t{tt}")
                    for j in range(nj):
                        pv(r, p, j)
                    finish_pair(r, p, idx)
                    budget = pump(min(budget + 2800, 4800))
                    continue
                for j in range(2, nj):
                    # scores(j) FIRST: it is on the exp-stream critical chain
                    # (slot freed by exp(j-2) -> scores(j) -> exp(j)); the PV
                    # pair's consumer is many js away, so it follows.
                    pstate[(r, p)]["pend"].append(scores(r, p, j))
                    pv(r, p, j - 2)
                    w = 512 - (128 * (j - 4 * r) if j - 4 * r > 0 else 0)
                    inc = (int(0.35 * w + 210) if pstate[(r, p)]["inline"]
                           else int(0.68 * w + 235))
                    budget = pump(min(budget + inc, 3600))
                # tail: drain last two PVs, prefilling the next pair's first
                # two score chunks in between so ACT never starves.
                if nxt is not None:
                    if nxt[0] != r:
                        drain_until(f"Q{nxt[0]}o0")
                    else:
                        # pair 1's prefill reads QT/KT o=1 of this range:
                        # force those chains out before emitting the reads.
                        drain_until(f"a{r}")
                    start_pair(*nxt)
                pv(r, p, nj - 2)
                if nxt is not None:
                    prefill_scores(*nxt, 0)
                pv(r, p, nj - 1)
                if nxt is not None:
                    prefill_scores(*nxt, 1)
                finish_pair(r, p, idx)
                # boundary window: the next pair's first two exps cover
                # ~2.2us of PE time - generous allowance drains a WO chain.
                budget = pump(min(budget + 2800, 4800))

            # Final drain: leftover fillers first, then the last range's WO
            # chains alternating pj/sc psum tags so FOUR slots keep the tail
            # pipeline deep. The first two chains' d=0 matmuls (pair-0 ctxT,
            # already normalized) are emitted up front so the PE stays busy
            # (and clocked up) through the final normalization latency.
            drain_until("__all__")
            chains = [
                wo_items(NT - 1, qq, o,
                         ptag=("pj" if (2 * qq + o) % 2 == 0 else "sc"))
                for qq in range(4) for o in range(2)
            ]
            for ch in chains[:2]:
                ch[0][1]()
            for ch in chains[:2]:
                ch[1][1]()
            for ch in chains[2:]:
                ch[0][1]()
                ch[1][1]()

    nc.compile()
    return nc


def _get_nc():
    if "nc" not in _CACHE:
        _CACHE["nc"] = _build()
    return _CACHE["nc"]


def kernel(x, Wq, Wk, Wv, Wo, bo):
    global LAST_RESULTS
    x = np.asarray(x, dtype=np.float32)
    Wq = np.asarray(Wq, dtype=np.float32)
    Wk = np.asarray(Wk, dtype=np.float32)
    Wv = np.asarray(Wv, dtype=np.float32)
    Wo = np.asarray(Wo, dtype=np.float32)
    bo = np.asarray(bo, dtype=np.float32)

    nc = _get_nc()
    # range-major layout: [NT*128, NC*512]; row 128*r+p holds chunks c=0..7
    # (512 tokens each, contiguous) of q-range r for feature-row p.

    def xarr(b):
        a = x[b].T.reshape(NC, 128, NT, 512).transpose(2, 1, 0, 3)
        return np.ascontiguousarray(a.reshape(NT * 128, NC * 512)).astype(NPDT)

    xTs = [xarr(b) for b in range(B)]

    def warr(w, cs):
        # [D, GW] slice -> [128, NC*GW]: partition p holds chunk-major rows
        s = w[:, cs].reshape(D // 128, 128, GW).transpose(1, 0, 2)
        return np.ascontiguousarray(s.reshape(128, -1)).astype(NPDT)

    def woarr(cs):
        # [GW, D] slice -> [128, 2*D]
        s = Wo[cs, :].reshape(GW // 128, 128, D).transpose(1, 0, 2)
        return np.ascontiguousarray(s.reshape(128, -1)).astype(NPDT)

    # causal mask block: keep (q - k >= 0) else -1e30  [partition=k, free=q]
    ktri = np.arange(128)
    tri_np = np.where(ktri[None, :] - ktri[:, None] >= 0, 0.0, NEG).astype(
        np.float32
    )

    in_maps = []
    for c in range(N_CORES):
        b, g = divmod(c, N_CORES // B)
        cs = slice(GW * g, GW * (g + 1))
        in_maps.append(
            {
                "xT": xTs[b],
                "wq": warr(Wq, cs),
                "wk": warr(Wk, cs),
                "wv": warr(Wv, cs),
                "wo": woarr(cs),
                "tri": tri_np,
            }
        )

    _maybe_install_trace_hook()
    res = bass_utils.run_bass_kernel_spmd(nc, in_maps, core_ids=list(range(N_CORES)))
    LAST_RESULTS = res

    out = np.zeros((B, S, D), dtype=np.float32)
    for c in range(N_CORES):
        out[c // (N_CORES // B)] += res.results[c]["out"].astype(np.float32)
    out += bo[None, None, :]
    return out
